# revision 5
# baseline (speedup 1.0000x reference)
"""Trainium2 Bass kernel for nn_DeepCPP (GAT + 2xGCN graph branch, conv1d seq
branch, fusion MLP), SPMD over 8 NeuronCores.

Sharding/strategy:
 - Nodes partitioned across cores in natural order (keeps sorted `batch`
   contiguous per core); within a core nodes are sorted by in-degree so
   128-node windows have near-uniform max degree (node-major slot grids),
   processed by segmented hardware loops.
 - GAT layer is gather-free: x[src] per edge slot is materialized host-side,
   attention logits computed on-device per slot-column via small matmuls, and
   exp(leakyrelu(a_s+a_d)) is factorized as max(P_e*T_d, R_e) with
   P=exp(a_s), R=exp(0.2*a_s), T=exp(0.8*a_d); the per-dst factor
   exp(-0.2*a_d) cancels in the softmax.
 - GCN layers gather fp16 rows (dinv-prescaled h) from an AllGathered table
   via one 2D-offset indirect DMA per 128-node window; aggregation is a
   strided vector reduction.
 - Mean-pool via one-hot selection matmuls into persistent PSUM, AllReduce of
   partials; seq branch runs first (replicated, feature-major) so it fills
   otherwise-idle engines during the graph phase.
"""

import sys

sys.path.insert(0, '/opt/trn_rl_repo')

import numpy as np
import ml_dtypes

import concourse.bass as bass
import concourse.mybir as mybir
import concourse.tile as tile
from concourse import bacc
from concourse.bass_utils import run_bass_kernel_spmd

F32 = mybir.dt.float32
F16 = mybir.dt.float16
BF16 = mybir.dt.bfloat16
I32 = mybir.dt.int32
AF = mybir.ActivationFunctionType
OP = mybir.AluOpType
AX = mybir.AxisListType

NC_CORES = 8
P = 128
DEBUG = False


# --------------------------------------------------------------------------
# host-side prep
# --------------------------------------------------------------------------

def _segments(Ts, max_segs=6):
    W = len(Ts)
    INF = float('inf')
    best = [[INF] * (max_segs + 1) for _ in range(W + 1)]
    arg = [[None] * (max_segs + 1) for _ in range(W + 1)]
    best[0][0] = 0.0
    for j in range(1, W + 1):
        for s in range(1, max_segs + 1):
            for i in range(j):
                if best[i][s - 1] == INF:
                    continue
                c = best[i][s - 1] + (j - i) * Ts[i]
                if c < best[j][s]:
                    best[j][s] = c
                    arg[j][s] = i
    s = min(range(1, max_segs + 1), key=lambda k: best[W][k])
    bounds = []
    j = W
    while j > 0:
        i = arg[j][s]
        bounds.append((i, j))
        j = i
        s -= 1
    bounds.reverse()
    return bounds


def host_prep(inputs):
    x = np.asarray(inputs['x'], np.float32)
    ei = np.asarray(inputs['edge_index'], np.int64)
    batch = np.asarray(inputs['batch'], np.int64)
    N = x.shape[0]
    Bsz = int(np.asarray(inputs['seq_data']).shape[0])
    assert N % NC_CORES == 0
    REAL = N // NC_CORES
    WPC = (REAL + P - 1) // P
    LOCAL = WPC * P
    NTOT = LOCAL * NC_CORES
    SENT = REAL if REAL < LOCAL else REAL - 1   # sentinel zero row in core 0

    src2 = np.concatenate([ei[0], np.arange(N)])
    dst2 = np.concatenate([ei[1], np.arange(N)])
    deg = np.bincount(dst2, minlength=N)

    local_rank = np.zeros(N, np.int64)
    rowid = np.zeros(N, np.int64)
    node_at = np.full((NC_CORES, LOCAL), -1, np.int64)
    for c in range(NC_CORES):
        ns = np.arange(c * REAL, (c + 1) * REAL)
        order = ns[np.argsort(-deg[ns], kind='stable')]
        local_rank[order] = np.arange(REAL)
        rowid[order] = c * LOCAL + np.arange(REAL)
        node_at[c, :REAL] = order

    Tw = np.ones(WPC, np.int64)
    for c in range(NC_CORES):
        first = node_at[c, ::P]
        for w in range(WPC):
            if first[w] >= 0:
                Tw[w] = max(Tw[w], deg[first[w]])
    segs = _segments([int(t) for t in Tw])
    seg_T = [int(Tw[w0]) for (w0, w1) in segs]
    col_off = np.zeros(WPC, np.int64)
    TW = np.zeros(WPC, np.int64)
    off = 0
    for (w0, w1), T in zip(segs, seg_T):
        for w in range(w0, w1):
            col_off[w] = off + (w - w0) * T
            TW[w] = T
        off += (w1 - w0) * T
    SLOTS = int(off)

    e_dst = rowid[dst2]
    e_src = src2
    o = np.argsort(e_dst, kind='stable')
    e_dst = e_dst[o]
    e_src = e_src[o]
    grp_start = np.searchsorted(e_dst, np.arange(NTOT), side='left')
    t_of = np.arange(len(e_dst)) - grp_start[e_dst]
    c_of = e_dst // LOCAL
    lrow = e_dst % LOCAL
    w_of = lrow // P
    p_of = lrow % P
    col = col_off[w_of] + t_of
    assert (t_of < TW[w_of]).all()

    slot_node = np.full((NC_CORES, P, SLOTS), N, np.int64)
    slot_node[c_of, p_of, col] = e_src

    x_pad = np.vstack([x, np.zeros((1, x.shape[1]), np.float32)])
    rowid_pad = np.concatenate([rowid, [SENT]]).astype(np.int32)

    cnt = np.bincount(batch, minlength=Bsz).astype(np.float32)
    per_core = []
    for c in range(NC_CORES):
        sn = slot_node[c]                          # [P, SLOTS], N = pad
        xs = x_pad[sn]                             # [P, SLOTS, 9]
        xslots = np.ascontiguousarray(xs.reshape(P, SLOTS * 9))
        xTl = np.zeros((16, SLOTS, P), np.float32)
        xTl[0:9] = xs.transpose(2, 1, 0)
        xTl[9] = (sn.T == N).astype(np.float32)    # pad flag
        xslotsT = np.ascontiguousarray(xTl.reshape(16, SLOTS * P))
        srcrow = rowid_pad[sn]

        valid = node_at[c] >= 0
        xloc = np.zeros((16, LOCAL), np.float32)
        xloc[0:9, valid] = x[node_at[c][valid]].T

        dg = np.full(LOCAL, 1e30, np.float32)
        dg[valid] = deg[node_at[c][valid]]
        deg_w = np.ascontiguousarray(dg.reshape(WPC, P).T)

        bl = np.full(LOCAL, -1.0, np.float32)
        b_base = int(batch[c * REAL])
        bl[valid] = batch[node_at[c][valid]] - b_base
        assert bl.max() < 256, "batch window exceeded 256"
        bl_w = np.ascontiguousarray(bl.reshape(WPC, P).T)

        cnt_l = np.ones(256, np.float32)
        hi = min(256, Bsz - b_base)
        cnt_l[:hi] = np.maximum(cnt[b_base:b_base + hi], 1.0)
        scatv = np.zeros(256, np.int32)
        for j in range(256):
            scatv[j] = b_base + j if b_base + j < Bsz else Bsz + (j % 8)

        per_core.append(dict(
            xslots=xslots, xslotsT=xslotsT, srcrow=srcrow.astype(np.int32),
            xlocT=xloc, deg_w=deg_w, bl_w=bl_w,
            cnt_l=np.ascontiguousarray(cnt_l.reshape(2, P).T),
            scat=np.ascontiguousarray(scatv.reshape(2, P).T),
        ))

    baked = dict(N=N, REAL=REAL, WPC=WPC, LOCAL=LOCAL, NTOT=NTOT,
                 SLOTS=SLOTS, segs=segs, seg_T=seg_T, Bsz=Bsz)
    return per_core, baked


def fold_weights(inputs):
    w = {k: np.asarray(v, np.float32) for k, v in inputs.items()
         if k not in ('x', 'edge_index', 'batch')}
    H, C = 4, 32
    Wg = w['W_gat']
    was = np.einsum('fhc,hc->fh', Wg.reshape(9, H, C), w['att_src'])
    wad = np.einsum('fhc,hc->fh', Wg.reshape(9, H, C), w['att_dst'])
    was_aug = np.zeros((16, 4), np.float32)
    was_aug[0:9] = was
    was_aug[9] = -80.0
    wad_aug = np.zeros((16, 4), np.float32)
    wad_aug[0:9] = wad
    wg_aug = np.zeros((128, 128), np.float32)
    for h in range(H):
        wg_aug[h * 32:h * 32 + 9, h * 32:(h + 1) * 32] = Wg[:, h * 32:(h + 1) * 32]
        wg_aug[h * 32 + 9, h * 32:(h + 1) * 32] = w['b_gat'][h * 32:(h + 1) * 32]
    W3_aug = np.zeros((65, 128), np.float32)
    W3_aug[0:64] = w['W3']
    W3_aug[64] = w['b3']

    def fold(cw, cb, g, be, m, v):
        s = g / np.sqrt(v + 1e-5)
        return cw * s[:, None, None], (cb - m) * s + be

    c1w, c1b = fold(w['conv1_w'], w['conv1_b'], w['bn1_g'], w['bn1_b'],
                    w['bn1_m'], w['bn1_v'])
    c2w, c2b = fold(w['conv2_w'], w['conv2_b'], w['bn2_g'], w['bn2_b'],
                    w['bn2_m'], w['bn2_v'])
    # [cin, k, cout] flattened so slice k -> [cin, cout]
    w1k = np.ascontiguousarray(c1w.transpose(1, 2, 0)).reshape(30, 3 * 64)
    w2k = np.ascontiguousarray(c2w.transpose(1, 2, 0)).reshape(64, 3 * 64)
    fc1_Wr = np.ascontiguousarray(w['fc1_W'].reshape(64, 16 * 64))

    seq = w['seq_data']                              # [B, 30, 20]
    xseq = np.ascontiguousarray(seq.transpose(1, 0, 2)).reshape(30, -1)

    return dict(
        was_aug=was_aug, wad_aug=wad_aug, wg_aug=wg_aug,
        W2=w['W2'], b2row=np.ascontiguousarray(np.broadcast_to(w['b2'], (P, 64))),
        W3_aug=W3_aug,
        w1k=w1k, b1=np.ascontiguousarray(c1b.reshape(64, 1)),
        w2k=w2k, b2c=np.ascontiguousarray(c2b.reshape(64, 1)),
        fc1_Wr=fc1_Wr, fc1_b=np.ascontiguousarray(w['fc1_b'].reshape(64, 1)),
        fus_W0=np.ascontiguousarray(w['fus_W'][0:128]),
        fus_W1=np.ascontiguousarray(w['fus_W'][128:192]),
        fus_b=np.ascontiguousarray(w['fus_b'].reshape(1, 128)),
        cls1_W=w['cls1_W'],
        cls1_b=np.ascontiguousarray(w['cls1_b'].reshape(1, 64)),
        cls3_W=w['cls3_W'],
        cls3_b_t=np.array([[float(w['cls3_b'][0])]], np.float32),
        xseq=xseq,
    )


# --------------------------------------------------------------------------
# device program
# --------------------------------------------------------------------------

def build_nc(baked, unroll=False):
    WPC, LOCAL, NTOT, SLOTS = (baked['WPC'], baked['LOCAL'], baked['NTOT'],
                               baked['SLOTS'])
    segs, seg_T = baked['segs'], baked['seg_T']
    Bsz = baked['Bsz']
    BROWS = Bsz + 8
    REALC = baked['REAL']
    RG = [list(range(NC_CORES))]

    nc = bacc.Bacc("TRN2", target_bir_lowering=False, debug=False,
                   num_devices=NC_CORES)

    def inp(name, shape, dt=F32):
        return nc.dram_tensor(name, shape, dt, kind="ExternalInput")

    xslots = inp("xslots", [P, SLOTS * 9])
    xslotsT = inp("xslotsT", [16, SLOTS * P])
    srcrow = inp("srcrow", [P, SLOTS], I32)
    xlocT = inp("xlocT", [16, LOCAL])
    deg_w = inp("deg_w", [P, WPC])
    bl_w = inp("bl_w", [P, WPC])
    cnt_l = inp("cnt_l", [P, 2])
    scat = inp("scat", [P, 2], I32)
    iota256 = inp("iota256", [P, 256])
    ident = inp("ident", [P, P])
    ones4 = inp("ones4", [P, 4])
    onesrow = inp("onesrow", [1, Bsz])
    was_aug = inp("was_aug", [16, 4])
    wad_aug = inp("wad_aug", [16, 4])
    wg_aug = inp("wg_aug", [128, 128])
    W2 = inp("W2", [128, 64])
    b2row = inp("b2row", [P, 64])
    W3_aug = inp("W3_aug", [65, 128])
    w1k = inp("w1k", [30, 3 * 64])
    b1 = inp("b1", [64, 1])
    w2k = inp("w2k", [64, 3 * 64])
    b2c = inp("b2c", [64, 1])
    fc1_Wr = inp("fc1_Wr", [64, 16 * 64])
    fc1_b = inp("fc1_b", [64, 1])
    fus_W0 = inp("fus_W0", [128, 128])
    fus_W1 = inp("fus_W1", [64, 128])
    fus_b = inp("fus_b", [1, 128])
    cls1_W = inp("cls1_W", [128, 64])
    cls1_b = inp("cls1_b", [1, 64])
    cls3_W = inp("cls3_W", [64, 1])
    cls3_b_t = inp("cls3_b_t", [1, 1])
    xseq = inp("xseq", [30, Bsz * 20])

    out = nc.dram_tensor("out", [1, Bsz], F32, kind="ExternalOutput")
    dbg_T2 = nc.dram_tensor("dbg_T2", [LOCAL, 64], F32, kind="ExternalOutput") if DEBUG else None
    dbg_T3 = nc.dram_tensor("dbg_T3", [LOCAL, 64], F32, kind="ExternalOutput") if DEBUG else None
    dbg_AR = nc.dram_tensor("dbg_AR", [BROWS, 128], F32, kind="ExternalOutput") if DEBUG else None
    dbg_sT = nc.dram_tensor("dbg_sT", [64, Bsz], F32, kind="ExternalOutput") if DEBUG else None

    T2_local = nc.dram_tensor("T2_local", [LOCAL, 64], F16)
    s1_dram = nc.dram_tensor("s1_dram", [64, Bsz * 18], F32)
    s2_dram = nc.dram_tensor("s2_dram", [64, Bsz * 16], F32)
    T2_full = nc.dram_tensor("T2_full", [NTOT, 64], F16)
    T3_local = nc.dram_tensor("T3_local", [LOCAL, 64], F16)
    T3_full = nc.dram_tensor("T3_full", [NTOT, 64], F16)
    AR_in = nc.dram_tensor("AR_in", [BROWS, 128], F32)
    AR_out = nc.dram_tensor("AR_out", [BROWS, 128], F32)

    def emit_loop(tc, w0, w1, body):
        if unroll:
            for w in range(w0, w1):
                body(w)
        else:
            with tc.For_i(w0, w1, 1) as w:
                body(w)

    with tile.TileContext(nc) as tc:
        with tc.tile_pool(name="const", bufs=1) as cp, \
             tc.tile_pool(name="work", bufs=2) as wp, \
             tc.tile_pool(name="gat", bufs=2) as gp, \
             tc.tile_pool(name="psum", bufs=4, space="PSUM") as pp, \
             tc.tile_pool(name="ppool", bufs=1, space="PSUM") as ppool, \
             tc.tile_pool(name="seq", bufs=1) as sq:

            def c_load(ap, shape, dt=F32):
                t = cp.tile(shape, dt, tag=f"c_{ap.name}")
                nc.sync.dma_start(t[:], ap[:])
                return t

            srcrow_sb = c_load(srcrow, [P, SLOTS], I32)
            deg_sb = c_load(deg_w, [P, WPC])
            bl_sb = c_load(bl_w, [P, WPC])
            cnt_sb = c_load(cnt_l, [P, 2])
            scat_sb = c_load(scat, [P, 2], I32)
            iota_sb = c_load(iota256, [P, 256])
            ident_sb = c_load(ident, [P, P])
            ones4_sb = c_load(ones4, [P, 4])
            was_sb = c_load(was_aug, [16, 4])
            wad_sb = c_load(wad_aug, [16, 4])
            wg_sb = c_load(wg_aug, [128, 128])
            W2_sb = c_load(W2, [128, 64])
            b2row_sb = c_load(b2row, [P, 64])
            W3_sb = c_load(W3_aug, [65, 128])

            dinv_sb = cp.tile([P, WPC], F32)
            nc.scalar.activation(dinv_sb[:], deg_sb[:], AF.Sqrt)
            nc.vector.reciprocal(dinv_sb[:], dinv_sb[:])

            # persistent pooling PSUM, zeroed via K=1 matmul (sets has_written)
            pool_ps0 = ppool.tile([P, P], F32, tag="pool0")
            pool_ps1 = ppool.tile([P, P], F32, tag="pool1")
            zrow = cp.tile([1, P], F32)
            nc.vector.memset(zrow[:], 0.0)
            nc.tensor.matmul(pool_ps0[:], zrow[:], zrow[:], start=True, stop=True)
            nc.tensor.matmul(pool_ps1[:], zrow[:], zrow[:], start=True, stop=True)

            # ================= seq branch (independent of graph; emitted
            # first so it fills PE/Act gaps during the Pool/DVE-heavy GAT) ==
            w1_sb = c_load(w1k, [30, 3 * 64])
            b1_sb = c_load(b1, [64, 1])
            w2_sb = c_load(w2k, [64, 3 * 64])
            b2c_sb = c_load(b2c, [64, 1])
            fc1_sb = c_load(fc1_Wr, [64, 16 * 64])
            fc1b_sb = c_load(fc1_b, [64, 1])
            fusW0_sb = c_load(fus_W0, [128, 128])
            fusW1_sb = c_load(fus_W1, [64, 128])
            fusb_sb = c_load(fus_b, [1, 128])
            cls1W_sb = c_load(cls1_W, [128, 64])
            cls1b_sb = c_load(cls1_b, [1, 64])
            cls3W_sb = c_load(cls3_W, [64, 1])
            cls3b_sb = c_load(cls3_b_t, [1, 1])
            onesr_sb = c_load(onesrow, [1, Bsz])

            CH1 = 28
            nb1 = (Bsz + CH1 - 1) // CH1
            for ci in range(nb1):
                b0 = ci * CH1
                bn = min(CH1, Bsz - b0)
                xs_ch = sq.tile([30, CH1 * 20], F32, tag="xs_ch")
                nc.sync.dma_start(xs_ch[:30, :bn * 20],
                                  xseq[:, b0 * 20:(b0 + bn) * 20])
                cps = pp.tile([64, CH1 * 18], F32, tag="ps")
                for k in range(3):
                    nc.tensor.matmul(
                        cps[:, :bn * 18],
                        w1_sb[:, 64 * k:64 * (k + 1)],
                        xs_ch[:].rearrange("c (b t) -> c b t", t=20)[:, 0:bn, k:k + 18],
                        start=(k == 0), stop=(k == 2))
                s1c = sq.tile([64, CH1 * 18], F32, tag="s1c")
                nc.scalar.activation(
                    s1c[:, :bn * 18], cps[:, :bn * 18],
                    AF.Lrelu, bias=b1_sb[:], alpha=0.01)
                nc.sync.dma_start(s1_dram[:, b0 * 18:(b0 + bn) * 18],
                                  s1c[:, :bn * 18])
            CH2 = 31
            nb2 = (Bsz + CH2 - 1) // CH2
            for ci in range(nb2):
                b0 = ci * CH2
                bn = min(CH2, Bsz - b0)
                s1c2 = sq.tile([64, CH2 * 18], F32, tag="s1c2")
                nc.sync.dma_start(s1c2[:, :bn * 18],
                                  s1_dram[:, b0 * 18:(b0 + bn) * 18])
                cps2 = pp.tile([64, CH2 * 16], F32, tag="ps")
                for k in range(3):
                    nc.tensor.matmul(
                        cps2[:, :bn * 16],
                        w2_sb[:, 64 * k:64 * (k + 1)],
                        s1c2[:].rearrange("c (b t) -> c b t", t=18)[:, 0:bn, k:k + 16],
                        start=(k == 0), stop=(k == 2))
                s2c = sq.tile([64, CH2 * 16], F32, tag="s2c")
                nc.scalar.activation(
                    s2c[:, :bn * 16], cps2[:, :bn * 16],
                    AF.Lrelu, bias=b2c_sb[:], alpha=0.01)
                nc.sync.dma_start(s2_dram[:, b0 * 16:(b0 + bn) * 16],
                                  s2c[:, :bn * 16])
            sT = sq.tile([64, Bsz], F32, tag="sT")
            for ci in range(Bsz // 512):
                b0 = ci * 512
                s2c3 = sq.tile([64, 512 * 16], F32, tag="s2c3")
                nc.sync.dma_start(s2c3[:], s2_dram[:, b0 * 16:(b0 + 512) * 16])
                fps = pp.tile([64, 512], F32, tag="ps")
                for t in range(16):
                    nc.tensor.matmul(
                        fps[:],
                        fc1_sb[:].rearrange("c (t j) -> c t j", j=64)[:, t, :],
                        s2c3[:].rearrange("c (b t) -> c b t", t=16)[:, :, t:t + 1],
                        start=(t == 0), stop=(t == 15))
                nc.scalar.activation(sT[:, b0:b0 + 512], fps[:],
                                     AF.Identity, bias=fc1b_sb[:])

            # ================= GAT =================
            def gat_body(w, w0, T, seg_col0):
                colb = seg_col0 - w0 * T
                xw = gp.tile([16, P], F32, tag="xw")
                nc.sync.dma_start(xw[:], xlocT[:, bass.ds(w * P, P)])
                ad_ps = pp.tile([P, 4], F32, tag="ps")
                nc.tensor.matmul(ad_ps[:], xw[:], wad_sb[:],
                                 start=True, stop=True)
                T_w = gp.tile([P, 4], F32, tag="Tw")
                nc.scalar.activation(T_w[:], ad_ps[:], AF.Exp, scale=0.8)

                XT = gp.tile([16, T * P], F32, tag="XT")
                nc.sync.dma_start(XT[:],
                                  xslotsT[:, bass.ds((colb + w * T) * P, T * P)])
                as_ps = pp.tile([P, 4 * T], F32, tag="ps")
                for t in range(T):
                    nc.tensor.matmul(as_ps[:, 4 * t:4 * t + 4],
                                     XT[:, t * P:(t + 1) * P], was_sb[:],
                                     start=True, stop=True)
                Pt = gp.tile([P, 4 * T], F32, tag="Pt")
                Rt = gp.tile([P, 4 * T], F32, tag="Rt")
                nc.scalar.activation(Pt[:], as_ps[:], AF.Exp, scale=1.0)
                nc.scalar.activation(Rt[:], as_ps[:], AF.Exp, scale=0.2)

                EX = gp.tile([P, 4 * T], F32, tag="EX")
                nc.vector.tensor_tensor(
                    EX[:].rearrange("p (t h) -> p t h", h=4),
                    Pt[:].rearrange("p (t h) -> p t h", h=4),
                    T_w[:, None, :].to_broadcast([P, T, 4]),
                    op=OP.mult)
                nc.vector.tensor_tensor(EX[:], EX[:], Rt[:], op=OP.max)
                S4 = gp.tile([P, 4], F32, tag="S4")
                nc.vector.tensor_reduce(
                    S4[:, :, None],
                    EX[:].rearrange("p (t h) -> p h t", h=4),
                    axis=AX.X, op=OP.add)
                nc.vector.reciprocal(S4[:], S4[:])
                AL = gp.tile([P, 4 * T], F32, tag="AL")
                nc.vector.tensor_tensor(
                    AL[:].rearrange("p (t h) -> p t h", h=4),
                    EX[:].rearrange("p (t h) -> p t h", h=4),
                    S4[:, None, :].to_broadcast([P, T, 4]),
                    op=OP.mult)

                XS = gp.tile([P, T * 9], F32, tag="XS")
                nc.sync.dma_start(XS[:],
                                  xslots[:, bass.ds((colb + w * T) * 9, T * 9)])
                ZR = gp.tile([P, T * 36], F32, tag="ZR")
                nc.vector.tensor_tensor(
                    ZR[:].rearrange("p (t h f) -> p t h f", h=4, f=9),
                    XS[:].rearrange("p (t f) -> p t f", f=9)[:, :, None, :]
                        .to_broadcast([P, T, 4, 9]),
                    AL[:].rearrange("p (t h) -> p t h", h=4)[:, :, :, None]
                        .to_broadcast([P, T, 4, 9]),
                    op=OP.mult)
                zaug = gp.tile([P, 128], F32, tag="zaug")
                nc.vector.memset(
                    zaug[:].rearrange("p (h t) -> p h t", t=32)[:, :, 10:32], 0.0)
                nc.vector.tensor_copy(
                    zaug[:].rearrange("p (h t) -> p h t", t=32)[:, :, 9:10],
                    ones4_sb[:, :, None])
                nc.vector.tensor_reduce(
                    zaug[:].rearrange("p (h t) -> p h t", t=32)[:, :, 0:9][:, :, :, None],
                    ZR[:].rearrange("p (t h f) -> p h f t", h=4, f=9),
                    axis=AX.X, op=OP.add)
                zT_ps = pp.tile([P, P], F32, tag="ps")
                nc.tensor.transpose(out=zT_ps[:], in_=zaug[:], identity=ident_sb[:])
                zT = gp.tile([P, P], F32, tag="zT")
                nc.scalar.copy(zT[:], zT_ps[:])
                g1_ps = pp.tile([P, P], F32, tag="ps")
                nc.tensor.matmul(g1_ps[:], wg_sb[:], zT[:],
                                 start=True, stop=True)
                g1T = gp.tile([P, P], F32, tag="g1T")
                nc.scalar.activation(g1T[:], g1_ps[:], AF.Lrelu, alpha=0.01)
                h2_ps = pp.tile([P, 64], F32, tag="ps")
                nc.tensor.matmul(h2_ps[:], g1T[:], W2_sb[:], start=True, stop=True)
                T2s = gp.tile([P, 64], F16, tag="T2s")
                nc.scalar.activation(T2s[:], h2_ps[:], AF.Copy,
                                     scale=dinv_sb[:, bass.ds(w, 1)])
                nc.sync.dma_start(T2_local[bass.ds(w * P, P), :], T2s[:])

            seg_col0 = 0
            for (w0, w1), T in zip(segs, seg_T):
                emit_loop(tc, w0, w1,
                          lambda w, w0=w0, T=T, s0=seg_col0: gat_body(w, w0, T, s0))
                seg_col0 += (w1 - w0) * T

            if LOCAL > REALC:
                ztail = wp.tile([LOCAL - REALC, 64], F16, tag="ztail")
                nc.vector.memset(ztail[:], 0.0)
                nc.sync.dma_start(T2_local[REALC:LOCAL, :], ztail[:])

            tc.strict_bb_all_engine_barrier()
            nc.gpsimd.collective_compute(
                "AllGather", OP.bypass, replica_groups=RG,
                ins=[T2_local.ap().opt()], outs=[T2_full.ap().opt()])
            tc.strict_bb_all_engine_barrier()

            # ================= GCN layers =================
            def gcn_body(w, w0, T, seg_col0, table, last):
                colb = seg_col0 - w0 * T
                IDXw = wp.tile([P, T], I32, tag="IDXw")
                nc.vector.tensor_copy(IDXw[:],
                                      srcrow_sb[:, bass.ds(colb + w * T, T)])
                G = wp.tile([P, T * 64], F16, tag="G")
                nc.gpsimd.indirect_dma_start(
                    out=G[:], out_offset=None,
                    in_=table[:],
                    in_offset=bass.IndirectOffsetOnAxis(
                        ap=IDXw[:, 0:T], axis=0))
                z = wp.tile([P, 64], F32, tag="z")
                nc.vector.tensor_reduce(
                    z[:, :, None],
                    G[:].rearrange("p (t c) -> p c t", c=64),
                    axis=AX.X, op=OP.add)
                if not last:
                    nc.vector.tensor_scalar(
                        z[:], z[:], dinv_sb[:, bass.ds(w, 1)], None, OP.mult)
                    nc.vector.tensor_tensor(z[:], z[:], b2row_sb[:], op=OP.add)
                    g2 = wp.tile([P, 64], F32, tag="g2")
                    nc.scalar.activation(g2[:], z[:], AF.Lrelu, alpha=0.01)
                    T3s = wp.tile([P, 64], F16, tag="T3s")
                    nc.scalar.activation(T3s[:], g2[:], AF.Copy,
                                         scale=dinv_sb[:, bass.ds(w, 1)])
                    nc.sync.dma_start(T3_local[bass.ds(w * P, P), :], T3s[:])
                else:
                    z3s = wp.tile([P, 65], F32, tag="z3s")
                    nc.scalar.activation(z3s[:, 0:64], z[:], AF.Copy,
                                         scale=dinv_sb[:, bass.ds(w, 1)])
                    nc.vector.tensor_copy(z3s[:, 64:65], ones4_sb[:, 0:1])
                    z3T_ps = pp.tile([65, P], F32, tag="ps")
                    nc.tensor.transpose(out=z3T_ps[:], in_=z3s[:],
                                        identity=ident_sb[:])
                    z3T = wp.tile([65, P], F32, tag="z3T")
                    nc.scalar.copy(z3T[:], z3T_ps[:])
                    g3_ps = pp.tile([P, P], F32, tag="ps")
                    nc.tensor.matmul(g3_ps[:], z3T[:], W3_sb[:],
                                     start=True, stop=True)
                    g3 = wp.tile([P, P], F32, tag="g3")
                    nc.scalar.activation(g3[:], g3_ps[:], AF.Lrelu, alpha=0.01)
                    Mp = wp.tile([P, 256], F32, tag="Mp")
                    nc.vector.tensor_scalar(
                        Mp[:], iota_sb[:], bl_sb[:, bass.ds(w, 1)], None,
                        OP.is_equal)
                    nc.tensor.matmul(pool_ps0[:], Mp[:, 0:128], g3[:],
                                     start=False, stop=True)
                    nc.tensor.matmul(pool_ps1[:], Mp[:, 128:256], g3[:],
                                     start=False, stop=True)

            seg_col0 = 0
            for (w0, w1), T in zip(segs, seg_T):
                emit_loop(tc, w0, w1,
                          lambda w, w0=w0, T=T, s0=seg_col0:
                          gcn_body(w, w0, T, s0, T2_full, False))
                seg_col0 += (w1 - w0) * T

            if LOCAL > REALC:
                ztail2 = wp.tile([LOCAL - REALC, 64], F16, tag="ztail")
                nc.vector.memset(ztail2[:], 0.0)
                nc.sync.dma_start(T3_local[REALC:LOCAL, :], ztail2[:])

            tc.strict_bb_all_engine_barrier()
            nc.gpsimd.collective_compute(
                "AllGather", OP.bypass, replica_groups=RG,
                ins=[T3_local.ap().opt()], outs=[T3_full.ap().opt()])
            tc.strict_bb_all_engine_barrier()

            seg_col0 = 0
            for (w0, w1), T in zip(segs, seg_T):
                emit_loop(tc, w0, w1,
                          lambda w, w0=w0, T=T, s0=seg_col0:
                          gcn_body(w, w0, T, s0, T3_full, True))
                seg_col0 += (w1 - w0) * T

            # ---- pool epilogue
            zb = wp.tile([P, 128], F32, tag="zb")
            nc.vector.memset(zb[:], 0.0)
            r0 = 0
            while r0 < BROWS:
                r1 = min(r0 + P, BROWS)
                nc.sync.dma_start(AR_in[r0:r1, :], zb[:r1 - r0, :])
                r0 = r1
            crec = wp.tile([P, 2], F32, tag="crec")
            nc.vector.reciprocal(crec[:], cnt_sb[:])
            for k, pps in enumerate((pool_ps0, pool_ps1)):
                pooled = wp.tile([P, 128], F32, tag="pooled")
                nc.scalar.activation(pooled[:], pps[:], AF.Copy,
                                     scale=crec[:, k:k + 1])
                nc.gpsimd.indirect_dma_start(
                    out=AR_in[:], out_offset=bass.IndirectOffsetOnAxis(
                        ap=scat_sb[:, k:k + 1], axis=0),
                    in_=pooled[:], in_offset=None)

            tc.strict_bb_all_engine_barrier()
            nc.gpsimd.collective_compute(
                "AllReduce", OP.add, replica_groups=RG,
                ins=[AR_in.ap().opt()], outs=[AR_out.ap().opt()])
            tc.strict_bb_all_engine_barrier()

            if DEBUG:
                dtile = sq.tile([P, 64], F16, tag="dtile")
                dtf = sq.tile([P, 64], F32, tag="dtf")
                for i in range(LOCAL // P):
                    nc.sync.dma_start(dtile[:], T2_local[i * P:(i + 1) * P, :])
                    nc.vector.tensor_copy(dtf[:], dtile[:])
                    nc.sync.dma_start(dbg_T2[i * P:(i + 1) * P, :], dtf[:])
                    nc.sync.dma_start(dtile[:], T3_local[i * P:(i + 1) * P, :])
                    nc.vector.tensor_copy(dtf[:], dtile[:])
                    nc.sync.dma_start(dbg_T3[i * P:(i + 1) * P, :], dtf[:])
                dtile2 = sq.tile([P, 128], F32, tag="dtile2")
                r0 = 0
                while r0 < BROWS:
                    r1 = min(r0 + P, BROWS)
                    nc.sync.dma_start(dtile2[:r1 - r0, :], AR_out[r0:r1, :])
                    nc.sync.dma_start(dbg_AR[r0:r1, :], dtile2[:r1 - r0, :])
                    r0 = r1
            poolT = sq.tile([P, Bsz], F32, tag="poolT")
            for i in range(Bsz // P):
                blk = sq.tile([P, P], F32, tag="blk")
                nc.sync.dma_start(blk[:], AR_out[i * P:(i + 1) * P, :])
                tp = pp.tile([P, P], F32, tag="ps")
                nc.tensor.transpose(out=tp[:], in_=blk[:], identity=ident_sb[:])
                nc.scalar.copy(poolT[:, i * P:(i + 1) * P], tp[:])

            # ---- fusion + classifier
            combT = sq.tile([P, Bsz], F32, tag="combT")
            for ci in range(Bsz // 512):
                b0 = ci * 512
                ups = pp.tile([P, 512], F32, tag="ps")
                nc.tensor.matmul(ups[:], fusW0_sb[:], poolT[:, b0:b0 + 512],
                                 start=True, stop=False)
                nc.tensor.matmul(ups[:], fusW1_sb[:], sT[:, b0:b0 + 512],
                                 start=False, stop=False)
                nc.tensor.matmul(ups[:], fusb_sb[:], onesr_sb[:, b0:b0 + 512],
                                 start=False, stop=True)
                nc.scalar.activation(combT[:, b0:b0 + 512], ups[:],
                                     AF.Lrelu, alpha=0.01)
            c1T = sq.tile([64, Bsz], F32, tag="c1T")
            for ci in range(Bsz // 512):
                b0 = ci * 512
                vps = pp.tile([64, 512], F32, tag="ps")
                nc.tensor.matmul(vps[:], cls1W_sb[:], combT[:, b0:b0 + 512],
                                 start=True, stop=False)
                nc.tensor.matmul(vps[:], cls1b_sb[:], onesr_sb[:, b0:b0 + 512],
                                 start=False, stop=True)
                nc.scalar.activation(c1T[:, b0:b0 + 512], vps[:],
                                     AF.Lrelu, alpha=0.01)
            out_sb = sq.tile([1, Bsz], F32, tag="out_sb")
            for ci in range(Bsz // 512):
                b0 = ci * 512
                ops_ = pp.tile([1, 512], F32, tag="ps")
                nc.tensor.matmul(ops_[:], cls3W_sb[:], c1T[:, b0:b0 + 512],
                                 start=True, stop=True)
                nc.vector.tensor_scalar(
                    out_sb[:, b0:b0 + 512], ops_[:], cls3b_sb[0:1, 0:1], None,
                    OP.add)
            if DEBUG:
                nc.sync.dma_start(dbg_sT[:], sT[:])
            nc.sync.dma_start(out[:], out_sb[:])

    nc.compile()
    return nc


# --------------------------------------------------------------------------
# entry point
# --------------------------------------------------------------------------

_CACHE = {}
_RUN_KW = {}      # test harness may set e.g. {'trace': True}
_LAST = [None]    # test harness reads BassKernelResults (exec_time_ns)


def kernel(**inputs):
    key = (np.asarray(inputs['edge_index']).tobytes(),)
    kh = hash(key)
    if kh not in _CACHE:
        per_core, baked = host_prep(inputs)
        nc = build_nc(baked)
        _CACHE[kh] = (per_core, baked, nc)
    per_core, baked, nc = _CACHE[kh]

    wts = fold_weights(inputs)
    Bsz = baked['Bsz']
    shared = dict(
        iota256=np.ascontiguousarray(
            np.broadcast_to(np.arange(256, dtype=np.float32), (P, 256))),
        ident=np.eye(P, dtype=np.float32),
        ones4=np.ones((P, 4), np.float32),
        onesrow=np.ones((1, Bsz), np.float32),
        **wts)
    in_maps = []
    for c in range(NC_CORES):
        m = dict(shared)
        m.update(per_core[c])
        in_maps.append(m)

    res = run_bass_kernel_spmd(nc, in_maps, core_ids=list(range(NC_CORES)),
                               **_RUN_KW)
    _LAST[0] = res
    o = res.results[0]["out"].reshape(Bsz, 1).astype(np.float32)
    return o


# revision 20
# speedup vs baseline: 1.2861x; 1.2861x over previous
"""Trainium2 Bass kernel for nn_DeepCPP (GAT + 2xGCN graph branch, conv1d seq
branch, fusion MLP), SPMD over 8 NeuronCores.

Sharding/strategy:
 - Nodes partitioned across cores in natural order (keeps sorted `batch`
   contiguous per core); within a core nodes are sorted by in-degree so
   128-node windows have near-uniform max degree (node-major slot grids),
   processed by segmented hardware loops.
 - GAT layer is gather-free: x[src] per edge slot is materialized host-side,
   attention logits computed on-device per slot-column via small matmuls, and
   exp(leakyrelu(a_s+a_d)) is factorized as max(P_e*T_d, R_e) with
   P=exp(a_s), R=exp(0.2*a_s), T=exp(0.8*a_d); the per-dst factor
   exp(-0.2*a_d) cancels in the softmax.
 - GCN layers gather fp16 rows (dinv-prescaled h) from an AllGathered table
   via one 2D-offset indirect DMA per 128-node window; aggregation is a
   strided vector reduction.
 - Mean-pool via one-hot selection matmuls into persistent PSUM, AllReduce of
   partials; seq branch runs first (replicated, feature-major) so it fills
   otherwise-idle engines during the graph phase.
"""

import sys

sys.path.insert(0, '/opt/trn_rl_repo')

import numpy as np
import ml_dtypes

import concourse.bass as bass
import concourse.mybir as mybir
import concourse.tile as tile
from concourse import bacc
from concourse.bass_utils import run_bass_kernel_spmd

F32 = mybir.dt.float32
F16 = mybir.dt.float16
BF16 = mybir.dt.bfloat16
I32 = mybir.dt.int32
AF = mybir.ActivationFunctionType
OP = mybir.AluOpType
AX = mybir.AxisListType

NC_CORES = 8
P = 128
DEBUG = False


# --------------------------------------------------------------------------
# host-side prep
# --------------------------------------------------------------------------

def _segments(Ts, max_segs=6):
    W = len(Ts)
    INF = float('inf')
    best = [[INF] * (max_segs + 1) for _ in range(W + 1)]
    arg = [[None] * (max_segs + 1) for _ in range(W + 1)]
    best[0][0] = 0.0
    for j in range(1, W + 1):
        for s in range(1, max_segs + 1):
            for i in range(j):
                if best[i][s - 1] == INF:
                    continue
                c = best[i][s - 1] + (j - i) * Ts[i]
                if c < best[j][s]:
                    best[j][s] = c
                    arg[j][s] = i
    s = min(range(1, max_segs + 1), key=lambda k: best[W][k])
    bounds = []
    j = W
    while j > 0:
        i = arg[j][s]
        bounds.append((i, j))
        j = i
        s -= 1
    bounds.reverse()
    return bounds


def host_prep(inputs):
    x = np.asarray(inputs['x'], np.float32)
    ei = np.asarray(inputs['edge_index'], np.int64)
    batch = np.asarray(inputs['batch'], np.int64)
    N = x.shape[0]
    Bsz = int(np.asarray(inputs['seq_data']).shape[0])
    assert N % NC_CORES == 0
    REAL = N // NC_CORES
    WPC = (REAL + P - 1) // P
    LOCAL = WPC * P
    NTOT = LOCAL * NC_CORES
    # Table rows live partition-major: node (core c, window w, partition p)
    # -> row c*LOCAL + p*WPC + w, so a window-group store is one strided DMA.
    assert REAL < LOCAL, "need at least one pad slot for the sentinel row"
    SENT = (P - 1) * WPC + (WPC - 1)            # always-zero pad row, core 0

    src2 = np.concatenate([ei[0], np.arange(N)])
    dst2 = np.concatenate([ei[1], np.arange(N)])
    deg = np.bincount(dst2, minlength=N)

    local_rank = np.zeros(N, np.int64)
    crow = np.zeros(N, np.int64)    # compute-layout row: c*LOCAL + w*P + p
    rowid = np.zeros(N, np.int64)   # table row (partition-major)
    node_at = np.full((NC_CORES, LOCAL), -1, np.int64)
    for c in range(NC_CORES):
        ns = np.arange(c * REAL, (c + 1) * REAL)
        order = ns[np.argsort(-deg[ns], kind='stable')]
        local_rank[order] = np.arange(REAL)
        crow[order] = c * LOCAL + np.arange(REAL)
        rowid[order] = (c * LOCAL + (np.arange(REAL) % P) * WPC
                        + np.arange(REAL) // P)
        node_at[c, :REAL] = order

    Tw = np.ones(WPC, np.int64)
    for c in range(NC_CORES):
        first = node_at[c, ::P]
        for w in range(WPC):
            if first[w] >= 0:
                Tw[w] = max(Tw[w], deg[first[w]])
    segs = _segments([int(t) for t in Tw])
    seg_T = [int(Tw[w0]) for (w0, w1) in segs]
    col_off = np.zeros(WPC, np.int64)
    TW = np.zeros(WPC, np.int64)
    off = 0
    for (w0, w1), T in zip(segs, seg_T):
        for w in range(w0, w1):
            col_off[w] = off + (w - w0) * T
            TW[w] = T
        off += (w1 - w0) * T
    SLOTS = int(off)

    e_dst = crow[dst2]
    e_src = src2
    o = np.argsort(e_dst, kind='stable')
    e_dst = e_dst[o]
    e_src = e_src[o]
    grp_start = np.searchsorted(e_dst, np.arange(NTOT), side='left')
    t_of = np.arange(len(e_dst)) - grp_start[e_dst]
    c_of = e_dst // LOCAL
    lrow = e_dst % LOCAL
    w_of = lrow // P
    p_of = lrow % P
    col = col_off[w_of] + t_of
    assert (t_of < TW[w_of]).all()

    slot_node = np.full((NC_CORES, P, SLOTS), N, np.int64)
    slot_node[c_of, p_of, col] = e_src

    x_pad = np.vstack([x, np.zeros((1, x.shape[1]), np.float32)])
    rowid_pad = np.concatenate([rowid, [SENT]]).astype(np.int32)

    cnt = np.bincount(batch, minlength=Bsz).astype(np.float32)
    per_core = []
    for c in range(NC_CORES):
        sn = slot_node[c]                          # [P, SLOTS], N = pad
        xs = x_pad[sn]                             # [P, SLOTS, 9]
        xslots = np.ascontiguousarray(xs.reshape(P, SLOTS * 9)).astype(np.float16)
        xTl = np.zeros((16, SLOTS, P), np.float32)
        xTl[0:9] = xs.transpose(2, 1, 0)
        xTl[9] = (sn.T == N).astype(np.float32)    # pad flag
        xslotsT = np.ascontiguousarray(xTl.reshape(16, SLOTS * P)).astype(np.float16)
        srcrow = rowid_pad[sn]

        valid = node_at[c] >= 0
        xloc = np.zeros((16, LOCAL), np.float32)
        xloc[0:9, valid] = x[node_at[c][valid]].T

        dg = np.full(LOCAL, 1e30, np.float32)
        dg[valid] = deg[node_at[c][valid]]
        deg_w = np.ascontiguousarray(dg.reshape(WPC, P).T)

        bl = np.full(LOCAL, -1.0, np.float32)
        b_base = int(batch[c * REAL])
        bl[valid] = batch[node_at[c][valid]] - b_base
        assert bl.max() < 256, "batch window exceeded 256"
        bl_w = np.ascontiguousarray(bl.reshape(WPC, P).T)

        cnt_l = np.ones(256, np.float32)
        hi = min(256, Bsz - b_base)
        cnt_l[:hi] = np.maximum(cnt[b_base:b_base + hi], 1.0)
        scatv = np.zeros(256, np.int32)
        for j in range(256):
            scatv[j] = b_base + j if b_base + j < Bsz else Bsz + (j % 8)

        per_core.append(dict(
            xslots=xslots, xslotsT=xslotsT, srcrow=srcrow.astype(np.int32),
            xlocT=xloc, deg_w=deg_w, bl_w=bl_w,
            cnt_l=np.ascontiguousarray(cnt_l.reshape(2, P).T),
            scat=np.ascontiguousarray(scatv.reshape(2, P).T),
        ))

    baked = dict(N=N, REAL=REAL, WPC=WPC, LOCAL=LOCAL, NTOT=NTOT,
                 SLOTS=SLOTS, segs=segs, seg_T=seg_T, Bsz=Bsz)
    return per_core, baked


def fold_weights(inputs):
    w = {k: np.asarray(v, np.float32) for k, v in inputs.items()
         if k not in ('x', 'edge_index', 'batch')}
    H, C = 4, 32
    Wg = w['W_gat']
    was = np.einsum('fhc,hc->fh', Wg.reshape(9, H, C), w['att_src'])
    wad = np.einsum('fhc,hc->fh', Wg.reshape(9, H, C), w['att_dst'])
    was_aug = np.zeros((16, 4), np.float32)
    was_aug[0:9] = was
    was_aug[9] = -80.0
    wad_aug = np.zeros((16, 4), np.float32)
    wad_aug[0:9] = wad
    wg_aug = np.zeros((128, 128), np.float32)
    for h in range(H):
        wg_aug[h * 32:h * 32 + 9, h * 32:(h + 1) * 32] = Wg[:, h * 32:(h + 1) * 32]
        wg_aug[h * 32 + 9, h * 32:(h + 1) * 32] = w['b_gat'][h * 32:(h + 1) * 32]
    W3_aug = np.zeros((65, 128), np.float32)
    W3_aug[0:64] = w['W3']
    W3_aug[64] = w['b3']

    def fold(cw, cb, g, be, m, v):
        s = g / np.sqrt(v + 1e-5)
        return cw * s[:, None, None], (cb - m) * s + be

    c1w, c1b = fold(w['conv1_w'], w['conv1_b'], w['bn1_g'], w['bn1_b'],
                    w['bn1_m'], w['bn1_v'])
    c2w, c2b = fold(w['conv2_w'], w['conv2_b'], w['bn2_g'], w['bn2_b'],
                    w['bn2_m'], w['bn2_v'])
    # [cin, k, cout] flattened so slice k -> [cin, cout]
    w1k = np.ascontiguousarray(c1w.transpose(1, 2, 0)).reshape(30, 3 * 64)
    w2k = np.ascontiguousarray(c2w.transpose(1, 2, 0)).reshape(64, 3 * 64)
    fc1_Wr = np.ascontiguousarray(w['fc1_W'].reshape(64, 16 * 64))

    seq = w['seq_data']                              # [B, 30, 20]
    xseq = np.ascontiguousarray(seq.transpose(1, 0, 2)).reshape(30, -1)

    return dict(
        was_aug=was_aug.astype(np.float16), wad_aug=wad_aug, wg_aug=wg_aug,
        W2=w['W2'], b2row=np.ascontiguousarray(np.broadcast_to(w['b2'], (P, 64))),
        W3_aug=W3_aug,
        w1k=w1k, b1=np.ascontiguousarray(c1b.reshape(64, 1)),
        w2k=w2k, b2c=np.ascontiguousarray(c2b.reshape(64, 1)),
        fc1_Wr=fc1_Wr, fc1_b=np.ascontiguousarray(w['fc1_b'].reshape(64, 1)),
        fus_W0=np.ascontiguousarray(w['fus_W'][0:128]),
        fus_W1=np.ascontiguousarray(w['fus_W'][128:192]),
        fus_b=np.ascontiguousarray(w['fus_b'].reshape(1, 128)),
        cls1_W=w['cls1_W'],
        cls1_b=np.ascontiguousarray(w['cls1_b'].reshape(1, 64)),
        cls3_W=w['cls3_W'],
        cls3_b_t=np.array([[float(w['cls3_b'][0])]], np.float32),
        xseq=xseq,
    )


# --------------------------------------------------------------------------
# device program
# --------------------------------------------------------------------------

def build_nc(baked, unroll=False):
    WPC, LOCAL, NTOT, SLOTS = (baked['WPC'], baked['LOCAL'], baked['NTOT'],
                               baked['SLOTS'])
    segs, seg_T = baked['segs'], baked['seg_T']
    Bsz = baked['Bsz']
    BROWS = Bsz + 8
    REALC = baked['REAL']
    RG = [list(range(NC_CORES))]

    nc = bacc.Bacc("TRN2", target_bir_lowering=False, debug=False,
                   num_devices=NC_CORES)

    def inp(name, shape, dt=F32):
        return nc.dram_tensor(name, shape, dt, kind="ExternalInput")

    xslots = inp("xslots", [P, SLOTS * 9], F16)
    xslotsT = inp("xslotsT", [16, SLOTS * P], F16)
    srcrow = inp("srcrow", [P, SLOTS], I32)
    xlocT = inp("xlocT", [16, LOCAL])
    deg_w = inp("deg_w", [P, WPC])
    bl_w = inp("bl_w", [P, WPC])
    cnt_l = inp("cnt_l", [P, 2])
    scat = inp("scat", [P, 2], I32)
    iota256 = inp("iota256", [P, 256])
    ident = inp("ident", [P, P])
    ones4 = inp("ones4", [P, 4])
    onesrow = inp("onesrow", [1, Bsz])
    was_aug = inp("was_aug", [16, 4], F16)
    wad_aug = inp("wad_aug", [16, 4])
    wg_aug = inp("wg_aug", [128, 128])
    W2 = inp("W2", [128, 64])
    b2row = inp("b2row", [P, 64])
    W3_aug = inp("W3_aug", [65, 128])
    w1k = inp("w1k", [30, 3 * 64])
    b1 = inp("b1", [64, 1])
    w2k = inp("w2k", [64, 3 * 64])
    b2c = inp("b2c", [64, 1])
    fc1_Wr = inp("fc1_Wr", [64, 16 * 64])
    fc1_b = inp("fc1_b", [64, 1])
    fus_W0 = inp("fus_W0", [128, 128])
    fus_W1 = inp("fus_W1", [64, 128])
    fus_b = inp("fus_b", [1, 128])
    cls1_W = inp("cls1_W", [128, 64])
    cls1_b = inp("cls1_b", [1, 64])
    cls3_W = inp("cls3_W", [64, 1])
    cls3_b_t = inp("cls3_b_t", [1, 1])
    xseq = inp("xseq", [30, Bsz * 20])

    out = nc.dram_tensor("out", [1, Bsz], F32, kind="ExternalOutput")
    dbg_T2 = nc.dram_tensor("dbg_T2", [LOCAL, 64], F32, kind="ExternalOutput") if DEBUG else None
    dbg_T3 = nc.dram_tensor("dbg_T3", [LOCAL, 64], F32, kind="ExternalOutput") if DEBUG else None
    dbg_AR = nc.dram_tensor("dbg_AR", [BROWS, 128], F32, kind="ExternalOutput") if DEBUG else None
    dbg_sT = nc.dram_tensor("dbg_sT", [64, Bsz], F32, kind="ExternalOutput") if DEBUG else None

    # Tables are partition-major: row (c*LOCAL + p*WPC + w) holds node
    # (core c, window w, partition p); T2_pm is the same buffer viewed
    # [P, WPC*64] so a G-window store is one strided DMA.
    T2_pm = nc.dram_tensor("T2_local", [P, WPC * 64], F16)
    s1_dram = nc.dram_tensor("s1_dram", [64, Bsz * 18], F32)
    s2_dram = nc.dram_tensor("s2_dram", [64, Bsz * 16], F32)
    T2_full = nc.dram_tensor("T2_full", [NTOT, 64], F16)
    T3_pm = nc.dram_tensor("T3_local", [P, WPC * 64], F16)
    T3_full = nc.dram_tensor("T3_full", [NTOT, 64], F16)
    AR_in = nc.dram_tensor("AR_in", [BROWS, 128], F32)
    AR_out = nc.dram_tensor("AR_out", [BROWS, 128], F32)

    def emit_grouped(tc, w0, w1, Gmax, body_group):
        """body_group(wb, G): emit G windows starting at window wb."""
        ngr = (w1 - w0) // Gmax
        if ngr > 0:
            if unroll:
                for wb in range(w0, w0 + ngr * Gmax, Gmax):
                    body_group(wb, Gmax)
            else:
                with tc.For_i(w0, w0 + ngr * Gmax, Gmax) as wb:
                    body_group(wb, Gmax)
        tail = (w1 - w0) % Gmax
        if tail:
            body_group(w0 + ngr * Gmax, tail)

    with tile.TileContext(nc) as tc:
        with tc.tile_pool(name="const", bufs=1) as cp, \
             tc.tile_pool(name="work", bufs=2) as wp, \
             tc.tile_pool(name="gat", bufs=2) as gp, \
             tc.tile_pool(name="psum", bufs=4, space="PSUM") as pp, \
             tc.tile_pool(name="pa", bufs=2, space="PSUM") as pa, \
             tc.tile_pool(name="ppool", bufs=1, space="PSUM") as ppool, \
             tc.tile_pool(name="seq", bufs=1) as sq:

            def c_load(ap, shape, dt=F32):
                t = cp.tile(shape, dt, tag=f"c_{ap.name}")
                nc.sync.dma_start(t[:], ap[:])
                return t

            srcrow_sb = c_load(srcrow, [P, SLOTS], I32)
            deg_sb = c_load(deg_w, [P, WPC])
            bl_sb = c_load(bl_w, [P, WPC])
            cnt_sb = c_load(cnt_l, [P, 2])
            scat_sb = c_load(scat, [P, 2], I32)
            iota_sb = c_load(iota256, [P, 256])
            ident_sb = c_load(ident, [P, P])
            ones4_sb = c_load(ones4, [P, 4])
            was_sb = c_load(was_aug, [16, 4], F16)
            wad_sb = c_load(wad_aug, [16, 4])
            wg_sb = c_load(wg_aug, [128, 128])
            W2_sb = c_load(W2, [128, 64])
            b2row_sb = c_load(b2row, [P, 64])
            W3_sb = c_load(W3_aug, [65, 128])

            dinv_sb = cp.tile([P, WPC], F32)
            nc.scalar.activation(dinv_sb[:], deg_sb[:], AF.Sqrt)
            nc.vector.reciprocal(dinv_sb[:], dinv_sb[:])

            # persistent pooling PSUM, zeroed via K=1 matmul (sets has_written)
            pool_ps0 = ppool.tile([P, P], F32, tag="pool0")
            pool_ps1 = ppool.tile([P, P], F32, tag="pool1")
            zrow = cp.tile([1, P], F32)
            nc.vector.memset(zrow[:], 0.0)
            nc.tensor.matmul(pool_ps0[:], zrow[:], zrow[:], start=True, stop=True)
            nc.tensor.matmul(pool_ps1[:], zrow[:], zrow[:], start=True, stop=True)

            # ================= seq branch (independent of graph; emitted
            # first so it fills PE/Act gaps during the Pool/DVE-heavy GAT) ==
            w1_sb = c_load(w1k, [30, 3 * 64])
            b1_sb = c_load(b1, [64, 1])
            w2_sb = c_load(w2k, [64, 3 * 64])
            b2c_sb = c_load(b2c, [64, 1])
            fc1_sb = c_load(fc1_Wr, [64, 16 * 64])
            fc1b_sb = c_load(fc1_b, [64, 1])
            fusW0_sb = c_load(fus_W0, [128, 128])
            fusW1_sb = c_load(fus_W1, [64, 128])
            fusb_sb = c_load(fus_b, [1, 128])
            cls1W_sb = c_load(cls1_W, [128, 64])
            cls1b_sb = c_load(cls1_b, [1, 64])
            cls3W_sb = c_load(cls3_W, [64, 1])
            cls3b_sb = c_load(cls3_b_t, [1, 1])
            onesr_sb = c_load(onesrow, [1, Bsz])

            CH1 = 28
            nb1 = (Bsz + CH1 - 1) // CH1
            for ci in range(nb1):
                b0 = ci * CH1
                bn = min(CH1, Bsz - b0)
                xs_ch = sq.tile([30, CH1 * 20], F32, tag="xs_ch")
                nc.sync.dma_start(xs_ch[:30, :bn * 20],
                                  xseq[:, b0 * 20:(b0 + bn) * 20])
                cps = pp.tile([64, CH1 * 18], F32, tag="ps")
                for k in range(3):
                    nc.tensor.matmul(
                        cps[:, :bn * 18],
                        w1_sb[:, 64 * k:64 * (k + 1)],
                        xs_ch[:].rearrange("c (b t) -> c b t", t=20)[:, 0:bn, k:k + 18],
                        start=(k == 0), stop=(k == 2))
                s1c = sq.tile([64, CH1 * 18], F32, tag="s1c")
                nc.scalar.activation(
                    s1c[:, :bn * 18], cps[:, :bn * 18],
                    AF.Lrelu, bias=b1_sb[:], alpha=0.01)
                nc.sync.dma_start(s1_dram[:, b0 * 18:(b0 + bn) * 18],
                                  s1c[:, :bn * 18])
            CH2 = 31
            nb2 = (Bsz + CH2 - 1) // CH2
            for ci in range(nb2):
                b0 = ci * CH2
                bn = min(CH2, Bsz - b0)
                s1c2 = sq.tile([64, CH2 * 18], F32, tag="s1c2")
                nc.sync.dma_start(s1c2[:, :bn * 18],
                                  s1_dram[:, b0 * 18:(b0 + bn) * 18])
                cps2 = pp.tile([64, CH2 * 16], F32, tag="ps")
                for k in range(3):
                    nc.tensor.matmul(
                        cps2[:, :bn * 16],
                        w2_sb[:, 64 * k:64 * (k + 1)],
                        s1c2[:].rearrange("c (b t) -> c b t", t=18)[:, 0:bn, k:k + 16],
                        start=(k == 0), stop=(k == 2))
                s2c = sq.tile([64, CH2 * 16], F32, tag="s2c")
                nc.scalar.activation(
                    s2c[:, :bn * 16], cps2[:, :bn * 16],
                    AF.Lrelu, bias=b2c_sb[:], alpha=0.01)
                nc.sync.dma_start(s2_dram[:, b0 * 16:(b0 + bn) * 16],
                                  s2c[:, :bn * 16])
            sT = sq.tile([64, Bsz], F32, tag="sT")
            for ci in range(Bsz // 512):
                b0 = ci * 512
                s2c3 = sq.tile([64, 512 * 16], F32, tag="s2c3")
                nc.sync.dma_start(s2c3[:], s2_dram[:, b0 * 16:(b0 + 512) * 16])
                fps = pp.tile([64, 512], F32, tag="ps")
                for t in range(16):
                    nc.tensor.matmul(
                        fps[:],
                        fc1_sb[:].rearrange("c (t j) -> c t j", j=64)[:, t, :],
                        s2c3[:].rearrange("c (b t) -> c b t", t=16)[:, :, t:t + 1],
                        start=(t == 0), stop=(t == 15))
                nc.scalar.activation(sT[:, b0:b0 + 512], fps[:],
                                     AF.Identity, bias=fc1b_sb[:])

            # ================= GAT =================
            def gat_group(wb, G, Gmax, T, seg_col0, w0):
                colb = seg_col0 - w0 * T
                xw = gp.tile([16, Gmax * P], F32, tag="xw")
                nc.scalar.dma_start(xw[:, :G * P], xlocT[:, bass.ds(wb * P, G * P)])
                ad_ps = pa.tile([P, 4 * Gmax], F32, tag="pa")
                for j in range(G):
                    nc.tensor.matmul(ad_ps[:, 4 * j:4 * j + 4],
                                     xw[:, j * P:(j + 1) * P], wad_sb[:],
                                     start=True, stop=True)
                T_w = gp.tile([P, 4 * Gmax], F32, tag="Tw")
                nc.scalar.activation(T_w[:, :4 * G], ad_ps[:, :4 * G],
                                     AF.Exp, scale=0.8)

                XT = gp.tile([16, Gmax * T * P], F16, tag="XT")
                nc.sync.dma_start(
                    XT[:, :G * T * P],
                    xslotsT[:, bass.ds((colb + wb * T) * P, G * T * P)])
                as_ps = pa.tile([P, 4 * Gmax * T], F32, tag="pa")
                for s in range(G * T):
                    nc.tensor.matmul(as_ps[:, 4 * s:4 * s + 4],
                                     XT[:, s * P:(s + 1) * P], was_sb[:],
                                     start=True, stop=True)
                Pt = gp.tile([P, 4 * Gmax * T], F32, tag="Pt")
                Rt = gp.tile([P, 4 * Gmax * T], F32, tag="Rt")
                nc.scalar.activation(Pt[:, :4 * G * T], as_ps[:, :4 * G * T],
                                     AF.Exp, scale=1.0)
                nc.scalar.activation(Rt[:, :4 * G * T], as_ps[:, :4 * G * T],
                                     AF.Exp, scale=0.2)

                XS = gp.tile([P, Gmax * T * 9], F16, tag="XS")
                nc.scalar.dma_start(
                    XS[:, :G * T * 9],
                    xslots[:, bass.ds((colb + wb * T) * 9, G * T * 9)])
                T2s_g = gp.tile([P, Gmax * 64], F16, tag="T2s")

                for j in range(G):
                    EX = gp.tile([P, 4 * T], F32, tag="EX")
                    nc.vector.tensor_tensor(
                        EX[:].rearrange("p (t h) -> p t h", h=4),
                        Pt[:, 4 * j * T:4 * (j + 1) * T]
                            .rearrange("p (t h) -> p t h", h=4),
                        T_w[:, 4 * j:4 * j + 4][:, None, :].to_broadcast([P, T, 4]),
                        op=OP.mult)
                    nc.vector.tensor_tensor(EX[:], EX[:],
                                            Rt[:, 4 * j * T:4 * (j + 1) * T],
                                            op=OP.max)
                    S4 = gp.tile([P, 4], F32, tag="S4")
                    nc.vector.tensor_reduce(
                        S4[:, :, None],
                        EX[:].rearrange("p (t h) -> p h t", h=4),
                        axis=AX.X, op=OP.add)
                    nc.vector.reciprocal(S4[:], S4[:])
                    AL = gp.tile([P, 4 * T], F16, tag="AL")
                    nc.vector.tensor_tensor(
                        AL[:].rearrange("p (t h) -> p t h", h=4),
                        EX[:].rearrange("p (t h) -> p t h", h=4),
                        S4[:, None, :].to_broadcast([P, T, 4]),
                        op=OP.mult)

                    ZR = gp.tile([P, T * 36], F16, tag="ZR")
                    nc.vector.tensor_tensor(
                        ZR[:].rearrange("p (t h f) -> p t h f", h=4, f=9),
                        XS[:, j * T * 9:(j + 1) * T * 9]
                            .rearrange("p (t f) -> p t f", f=9)[:, :, None, :]
                            .to_broadcast([P, T, 4, 9]),
                        AL[:].rearrange("p (t h) -> p t h", h=4)[:, :, :, None]
                            .to_broadcast([P, T, 4, 9]),
                        op=OP.mult)
                    zaug = gp.tile([P, 128], F32, tag="zaug")
                    nc.vector.memset(
                        zaug[:].rearrange("p (h t) -> p h t", t=32)[:, :, 10:32], 0.0)
                    nc.vector.tensor_copy(
                        zaug[:].rearrange("p (h t) -> p h t", t=32)[:, :, 9:10],
                        ones4_sb[:, :, None])
                    nc.vector.tensor_reduce(
                        zaug[:].rearrange("p (h t) -> p h t", t=32)[:, :, 0:9][:, :, :, None],
                        ZR[:].rearrange("p (t h f) -> p h f t", h=4, f=9),
                        axis=AX.X, op=OP.add)
                    zT_ps = pp.tile([P, P], F32, tag="ps")
                    nc.tensor.transpose(out=zT_ps[:], in_=zaug[:],
                                        identity=ident_sb[:])
                    zT = gp.tile([P, P], F32, tag="zT")
                    nc.scalar.copy(zT[:], zT_ps[:])
                    g1_ps = pp.tile([P, P], F32, tag="ps")
                    nc.tensor.matmul(g1_ps[:], wg_sb[:], zT[:],
                                     start=True, stop=True)
                    g1T = gp.tile([P, P], F32, tag="g1T")
                    nc.scalar.activation(g1T[:], g1_ps[:], AF.Lrelu, alpha=0.01)
                    h2_ps = pp.tile([P, 64], F32, tag="ps")
                    nc.tensor.matmul(h2_ps[:], g1T[:], W2_sb[:],
                                     start=True, stop=True)
                    nc.scalar.activation(T2s_g[:, j * 64:(j + 1) * 64], h2_ps[:],
                                         AF.Copy,
                                         scale=dinv_sb[:, bass.ds(wb + j, 1)])
                nc.sync.dma_start(T2_pm[:, bass.ds(wb * 64, G * 64)],
                                  T2s_g[:, :G * 64])

            def run_gat():
                seg_col0 = 0
                for (w0, w1), T in zip(segs, seg_T):
                    Gmax = max(1, min(4, 512 // (4 * T)))
                    emit_grouped(
                        tc, w0, w1, Gmax,
                        lambda wb, G, Gmax=Gmax, T=T, s0=seg_col0, w0=w0:
                        gat_group(wb, G, Gmax, T, s0, w0))
                    seg_col0 += (w1 - w0) * T
            run_gat()

            lastw = REALC // P
            pstart = REALC % P
            ztail = wp.tile([P - pstart, 64], F16, tag="ztail")
            nc.vector.memset(ztail[:], 0.0)
            nc.sync.dma_start(T2_pm[pstart:P, bass.ds(lastw * 64, 64)], ztail[:])

            tc.strict_bb_all_engine_barrier()
            nc.gpsimd.collective_compute(
                "AllGather", OP.bypass, replica_groups=RG,
                ins=[T2_pm.ap().opt()], outs=[T2_full.ap().opt()])
            tc.strict_bb_all_engine_barrier()

            # ================= GCN layers =================
            def gcn_group(wb, G, Gmax, T, seg_col0, w0, table, last):
                colb = seg_col0 - w0 * T
                IDX = wp.tile([P, Gmax * T], I32, tag="IDXw")
                nc.vector.tensor_copy(IDX[:, :G * T],
                                      srcrow_sb[:, bass.ds(colb + wb * T, G * T)])
                Gt = wp.tile([P, Gmax * T * 64], F16, tag="G")
                nc.gpsimd.indirect_dma_start(
                    out=Gt[:, :G * T * 64], out_offset=None,
                    in_=table[:],
                    in_offset=bass.IndirectOffsetOnAxis(
                        ap=IDX[:, 0:G * T], axis=0))
                if not last:
                    T3s_g = wp.tile([P, Gmax * 64], F16, tag="T3s")
                for j in range(G):
                    z = wp.tile([P, 64], F32, tag="z")
                    nc.vector.tensor_reduce(
                        z[:, :, None],
                        Gt[:, j * T * 64:(j + 1) * T * 64]
                            .rearrange("p (t c) -> p c t", c=64),
                        axis=AX.X, op=OP.add)
                    if not last:
                        nc.vector.tensor_scalar(
                            z[:], z[:], dinv_sb[:, bass.ds(wb + j, 1)], None,
                            OP.mult)
                        nc.vector.tensor_tensor(z[:], z[:], b2row_sb[:], op=OP.add)
                        g2 = wp.tile([P, 64], F32, tag="g2")
                        nc.scalar.activation(g2[:], z[:], AF.Lrelu, alpha=0.01)
                        nc.scalar.activation(T3s_g[:, j * 64:(j + 1) * 64], g2[:],
                                             AF.Copy,
                                             scale=dinv_sb[:, bass.ds(wb + j, 1)])
                    else:
                        z3s = wp.tile([P, 65], F32, tag="z3s")
                        nc.scalar.activation(z3s[:, 0:64], z[:], AF.Copy,
                                             scale=dinv_sb[:, bass.ds(wb + j, 1)])
                        nc.vector.tensor_copy(z3s[:, 64:65], ones4_sb[:, 0:1])
                        z3T_ps = pp.tile([65, P], F32, tag="ps")
                        nc.tensor.transpose(out=z3T_ps[:], in_=z3s[:],
                                            identity=ident_sb[:])
                        z3T = wp.tile([65, P], F32, tag="z3T")
                        nc.scalar.copy(z3T[:], z3T_ps[:])
                        g3_ps = pp.tile([P, P], F32, tag="ps")
                        nc.tensor.matmul(g3_ps[:], z3T[:], W3_sb[:],
                                         start=True, stop=True)
                        g3 = wp.tile([P, P], F32, tag="g3")
                        nc.scalar.activation(g3[:], g3_ps[:], AF.Lrelu, alpha=0.01)
                        Mp = wp.tile([P, 256], F32, tag="Mp")
                        nc.vector.tensor_scalar(
                            Mp[:], iota_sb[:], bl_sb[:, bass.ds(wb + j, 1)], None,
                            OP.is_equal)
                        nc.tensor.matmul(pool_ps0[:], Mp[:, 0:128], g3[:],
                                         start=False, stop=True)
                        nc.tensor.matmul(pool_ps1[:], Mp[:, 128:256], g3[:],
                                         start=False, stop=True)
                if not last:
                    nc.scalar.dma_start(T3_pm[:, bass.ds(wb * 64, G * 64)],
                                        T3s_g[:, :G * 64])

            def run_gcn(table, last):
                seg_col0 = 0
                for (w0, w1), T in zip(segs, seg_T):
                    Gmax = max(1, min(4, 512 // (4 * T)))
                    emit_grouped(
                        tc, w0, w1, Gmax,
                        lambda wb, G, Gmax=Gmax, T=T, s0=seg_col0, w0=w0:
                        gcn_group(wb, G, Gmax, T, s0, w0, table, last))
                    seg_col0 += (w1 - w0) * T

            run_gcn(T2_full, False)

            ztail2 = wp.tile([P - pstart, 64], F16, tag="ztail")
            nc.vector.memset(ztail2[:], 0.0)
            nc.sync.dma_start(T3_pm[pstart:P, bass.ds(lastw * 64, 64)], ztail2[:])

            tc.strict_bb_all_engine_barrier()
            nc.gpsimd.collective_compute(
                "AllGather", OP.bypass, replica_groups=RG,
                ins=[T3_pm.ap().opt()], outs=[T3_full.ap().opt()])
            tc.strict_bb_all_engine_barrier()

            run_gcn(T3_full, True)

            # ---- pool epilogue
            zb = wp.tile([P, 128], F32, tag="zb")
            nc.vector.memset(zb[:], 0.0)
            r0 = 0
            while r0 < BROWS:
                r1 = min(r0 + P, BROWS)
                nc.sync.dma_start(AR_in[r0:r1, :], zb[:r1 - r0, :])
                r0 = r1
            crec = wp.tile([P, 2], F32, tag="crec")
            nc.vector.reciprocal(crec[:], cnt_sb[:])
            for k, pps in enumerate((pool_ps0, pool_ps1)):
                pooled = wp.tile([P, 128], F32, tag="pooled")
                nc.scalar.activation(pooled[:], pps[:], AF.Copy,
                                     scale=crec[:, k:k + 1])
                nc.gpsimd.indirect_dma_start(
                    out=AR_in[:], out_offset=bass.IndirectOffsetOnAxis(
                        ap=scat_sb[:, k:k + 1], axis=0),
                    in_=pooled[:], in_offset=None)

            tc.strict_bb_all_engine_barrier()
            nc.gpsimd.collective_compute(
                "AllReduce", OP.add, replica_groups=RG,
                ins=[AR_in.ap().opt()], outs=[AR_out.ap().opt()])
            tc.strict_bb_all_engine_barrier()

            if DEBUG:
                dtile2 = sq.tile([P, 128], F32, tag="dtile2")
                r0 = 0
                while r0 < BROWS:
                    r1 = min(r0 + P, BROWS)
                    nc.sync.dma_start(dtile2[:r1 - r0, :], AR_out[r0:r1, :])
                    nc.sync.dma_start(dbg_AR[r0:r1, :], dtile2[:r1 - r0, :])
                    r0 = r1
            poolT = sq.tile([P, Bsz], F32, tag="poolT")
            for i in range(Bsz // P):
                blk = sq.tile([P, P], F32, tag="blk")
                nc.sync.dma_start(blk[:], AR_out[i * P:(i + 1) * P, :])
                tp = pp.tile([P, P], F32, tag="ps")
                nc.tensor.transpose(out=tp[:], in_=blk[:], identity=ident_sb[:])
                nc.scalar.copy(poolT[:, i * P:(i + 1) * P], tp[:])

            # ---- fusion + classifier
            combT = sq.tile([P, Bsz], F32, tag="combT")
            for ci in range(Bsz // 512):
                b0 = ci * 512
                ups = pp.tile([P, 512], F32, tag="ps")
                nc.tensor.matmul(ups[:], fusW0_sb[:], poolT[:, b0:b0 + 512],
                                 start=True, stop=False)
                nc.tensor.matmul(ups[:], fusW1_sb[:], sT[:, b0:b0 + 512],
                                 start=False, stop=False)
                nc.tensor.matmul(ups[:], fusb_sb[:], onesr_sb[:, b0:b0 + 512],
                                 start=False, stop=True)
                nc.scalar.activation(combT[:, b0:b0 + 512], ups[:],
                                     AF.Lrelu, alpha=0.01)
            c1T = sq.tile([64, Bsz], F32, tag="c1T")
            for ci in range(Bsz // 512):
                b0 = ci * 512
                vps = pp.tile([64, 512], F32, tag="ps")
                nc.tensor.matmul(vps[:], cls1W_sb[:], combT[:, b0:b0 + 512],
                                 start=True, stop=False)
                nc.tensor.matmul(vps[:], cls1b_sb[:], onesr_sb[:, b0:b0 + 512],
                                 start=False, stop=True)
                nc.scalar.activation(c1T[:, b0:b0 + 512], vps[:],
                                     AF.Lrelu, alpha=0.01)
            out_sb = sq.tile([1, Bsz], F32, tag="out_sb")
            for ci in range(Bsz // 512):
                b0 = ci * 512
                ops_ = pp.tile([1, 512], F32, tag="ps")
                nc.tensor.matmul(ops_[:], cls3W_sb[:], c1T[:, b0:b0 + 512],
                                 start=True, stop=True)
                nc.vector.tensor_scalar(
                    out_sb[:, b0:b0 + 512], ops_[:], cls3b_sb[0:1, 0:1], None,
                    OP.add)
            if DEBUG:
                nc.sync.dma_start(dbg_sT[:], sT[:])
            nc.sync.dma_start(out[:], out_sb[:])

    nc.compile()
    return nc


# --------------------------------------------------------------------------
# entry point
# --------------------------------------------------------------------------

_CACHE = {}
_RUN_KW = {}      # test harness may set e.g. {'trace': True}
_LAST = [None]    # test harness reads BassKernelResults (exec_time_ns)


def kernel(**inputs):
    key = (np.asarray(inputs['edge_index']).tobytes(),)
    kh = hash(key)
    if kh not in _CACHE:
        per_core, baked = host_prep(inputs)
        nc = build_nc(baked)
        _CACHE[kh] = (per_core, baked, nc)
    per_core, baked, nc = _CACHE[kh]

    wts = fold_weights(inputs)
    Bsz = baked['Bsz']
    shared = dict(
        iota256=np.ascontiguousarray(
            np.broadcast_to(np.arange(256, dtype=np.float32), (P, 256))),
        ident=np.eye(P, dtype=np.float32),
        ones4=np.ones((P, 4), np.float32),
        onesrow=np.ones((1, Bsz), np.float32),
        **wts)
    in_maps = []
    for c in range(NC_CORES):
        m = dict(shared)
        m.update(per_core[c])
        in_maps.append(m)

    res = run_bass_kernel_spmd(nc, in_maps, core_ids=list(range(NC_CORES)),
                               **_RUN_KW)
    _LAST[0] = res
    o = res.results[0]["out"].reshape(Bsz, 1).astype(np.float32)
    return o


# revision 21
# speedup vs baseline: 1.5223x; 1.1837x over previous
"""Trainium2 Bass kernel for nn_DeepCPP (GAT + 2xGCN graph branch, conv1d seq
branch, fusion MLP), SPMD over 8 NeuronCores.

Sharding/strategy:
 - Nodes partitioned across cores in natural order (keeps sorted `batch`
   contiguous per core); within a core nodes are sorted by in-degree so
   128-node windows have near-uniform max degree (node-major slot grids),
   processed by segmented hardware loops.
 - GAT layer is gather-free: x[src] per edge slot is materialized host-side,
   attention logits computed on-device per slot-column via small matmuls, and
   exp(leakyrelu(a_s+a_d)) is factorized as max(P_e*T_d, R_e) with
   P=exp(a_s), R=exp(0.2*a_s), T=exp(0.8*a_d); the per-dst factor
   exp(-0.2*a_d) cancels in the softmax.
 - GCN layers gather fp16 rows (dinv-prescaled h) from an AllGathered table
   via one 2D-offset indirect DMA per 128-node window; aggregation is a
   strided vector reduction.
 - Mean-pool via one-hot selection matmuls into persistent PSUM, AllReduce of
   partials; seq branch runs first (replicated, feature-major) so it fills
   otherwise-idle engines during the graph phase.
"""

import sys

sys.path.insert(0, '/opt/trn_rl_repo')

import numpy as np
import ml_dtypes

import concourse.bass as bass
import concourse.mybir as mybir
import concourse.tile as tile
from concourse import bacc
from concourse.bass_utils import run_bass_kernel_spmd

F32 = mybir.dt.float32
F16 = mybir.dt.float16
BF16 = mybir.dt.bfloat16
I32 = mybir.dt.int32
AF = mybir.ActivationFunctionType
OP = mybir.AluOpType
AX = mybir.AxisListType

NC_CORES = 8
P = 128
DEBUG = False


# --------------------------------------------------------------------------
# host-side prep
# --------------------------------------------------------------------------

def _segments(Ts, max_segs=6):
    W = len(Ts)
    INF = float('inf')
    best = [[INF] * (max_segs + 1) for _ in range(W + 1)]
    arg = [[None] * (max_segs + 1) for _ in range(W + 1)]
    best[0][0] = 0.0
    for j in range(1, W + 1):
        for s in range(1, max_segs + 1):
            for i in range(j):
                if best[i][s - 1] == INF:
                    continue
                c = best[i][s - 1] + (j - i) * Ts[i]
                if c < best[j][s]:
                    best[j][s] = c
                    arg[j][s] = i
    s = min(range(1, max_segs + 1), key=lambda k: best[W][k])
    bounds = []
    j = W
    while j > 0:
        i = arg[j][s]
        bounds.append((i, j))
        j = i
        s -= 1
    bounds.reverse()
    return bounds


def host_prep(inputs):
    x = np.asarray(inputs['x'], np.float32)
    ei = np.asarray(inputs['edge_index'], np.int64)
    batch = np.asarray(inputs['batch'], np.int64)
    N = x.shape[0]
    Bsz = int(np.asarray(inputs['seq_data']).shape[0])
    assert N % NC_CORES == 0
    REAL = N // NC_CORES
    WPC = (REAL + P - 1) // P
    LOCAL = WPC * P
    NTOT = LOCAL * NC_CORES
    # Table rows live partition-major: node (core c, window w, partition p)
    # -> row c*LOCAL + p*WPC + w, so a window-group store is one strided DMA.
    assert REAL < LOCAL, "need at least one pad slot for the sentinel row"
    SENT = (P - 1) * WPC + (WPC - 1)            # always-zero pad row, core 0

    src2 = np.concatenate([ei[0], np.arange(N)])
    dst2 = np.concatenate([ei[1], np.arange(N)])
    deg = np.bincount(dst2, minlength=N)

    local_rank = np.zeros(N, np.int64)
    crow = np.zeros(N, np.int64)    # compute-layout row: c*LOCAL + w*P + p
    rowid = np.zeros(N, np.int64)   # table row (partition-major)
    node_at = np.full((NC_CORES, LOCAL), -1, np.int64)
    for c in range(NC_CORES):
        ns = np.arange(c * REAL, (c + 1) * REAL)
        order = ns[np.argsort(-deg[ns], kind='stable')]
        local_rank[order] = np.arange(REAL)
        crow[order] = c * LOCAL + np.arange(REAL)
        rowid[order] = (c * LOCAL + (np.arange(REAL) % P) * WPC
                        + np.arange(REAL) // P)
        node_at[c, :REAL] = order

    Tw = np.ones(WPC, np.int64)
    for c in range(NC_CORES):
        first = node_at[c, ::P]
        for w in range(WPC):
            if first[w] >= 0:
                Tw[w] = max(Tw[w], deg[first[w]])
    segs = _segments([int(t) for t in Tw])
    seg_T = [int(Tw[w0]) for (w0, w1) in segs]
    col_off = np.zeros(WPC, np.int64)
    TW = np.zeros(WPC, np.int64)
    off = 0
    for (w0, w1), T in zip(segs, seg_T):
        for w in range(w0, w1):
            col_off[w] = off + (w - w0) * T
            TW[w] = T
        off += (w1 - w0) * T
    SLOTS = int(off)

    e_dst = crow[dst2]
    e_src = src2
    o = np.argsort(e_dst, kind='stable')
    e_dst = e_dst[o]
    e_src = e_src[o]
    grp_start = np.searchsorted(e_dst, np.arange(NTOT), side='left')
    t_of = np.arange(len(e_dst)) - grp_start[e_dst]
    c_of = e_dst // LOCAL
    lrow = e_dst % LOCAL
    w_of = lrow // P
    p_of = lrow % P
    col = col_off[w_of] + t_of
    assert (t_of < TW[w_of]).all()

    slot_node = np.full((NC_CORES, P, SLOTS), N, np.int64)
    slot_node[c_of, p_of, col] = e_src

    x_pad = np.vstack([x, np.zeros((1, x.shape[1]), np.float32)])
    rowid_pad = np.concatenate([rowid, [SENT]]).astype(np.int32)

    cnt = np.bincount(batch, minlength=Bsz).astype(np.float32)
    per_core = []
    for c in range(NC_CORES):
        sn = slot_node[c]                          # [P, SLOTS], N = pad
        xs = x_pad[sn]                             # [P, SLOTS, 9]
        xslots = np.ascontiguousarray(xs.reshape(P, SLOTS * 9)).astype(np.float16)
        xTl = np.zeros((16, SLOTS, P), np.float32)
        xTl[0:9] = xs.transpose(2, 1, 0)
        xTl[9] = (sn.T == N).astype(np.float32)    # pad flag
        xslotsT = np.ascontiguousarray(xTl.reshape(16, SLOTS * P)).astype(np.float16)
        srcrow = rowid_pad[sn]

        valid = node_at[c] >= 0
        xloc = np.zeros((16, LOCAL), np.float32)
        xloc[0:9, valid] = x[node_at[c][valid]].T

        dg = np.full(LOCAL, 1e30, np.float32)
        dg[valid] = deg[node_at[c][valid]]
        deg_w = np.ascontiguousarray(dg.reshape(WPC, P).T)

        bl = np.full(LOCAL, -1.0, np.float32)
        b_base = int(batch[c * REAL])
        bl[valid] = batch[node_at[c][valid]] - b_base
        assert bl.max() < 256, "batch window exceeded 256"
        bl_w = np.ascontiguousarray(bl.reshape(WPC, P).T)

        cnt_l = np.ones(256, np.float32)
        hi = min(256, Bsz - b_base)
        cnt_l[:hi] = np.maximum(cnt[b_base:b_base + hi], 1.0)
        scatv = np.zeros(256, np.int32)
        for j in range(256):
            scatv[j] = b_base + j if b_base + j < Bsz else Bsz + (j % 8)

        per_core.append(dict(
            xslots=xslots, xslotsT=xslotsT, srcrow=srcrow.astype(np.int32),
            xlocT=xloc, deg_w=deg_w, bl_w=bl_w,
            cnt_l=np.ascontiguousarray(cnt_l.reshape(2, P).T),
            scat=np.ascontiguousarray(scatv.reshape(2, P).T),
        ))

    baked = dict(N=N, REAL=REAL, WPC=WPC, LOCAL=LOCAL, NTOT=NTOT,
                 SLOTS=SLOTS, segs=segs, seg_T=seg_T, Bsz=Bsz)
    return per_core, baked


def fold_weights(inputs):
    w = {k: np.asarray(v, np.float32) for k, v in inputs.items()
         if k not in ('x', 'edge_index', 'batch')}
    H, C = 4, 32
    Wg = w['W_gat']
    was = np.einsum('fhc,hc->fh', Wg.reshape(9, H, C), w['att_src'])
    wad = np.einsum('fhc,hc->fh', Wg.reshape(9, H, C), w['att_dst'])
    was_aug = np.zeros((16, 4), np.float32)
    was_aug[0:9] = was
    was_aug[9] = -80.0
    wad_aug = np.zeros((16, 4), np.float32)
    wad_aug[0:9] = wad
    wg_aug = np.zeros((128, 128), np.float32)
    for h in range(H):
        wg_aug[h * 32:h * 32 + 9, h * 32:(h + 1) * 32] = Wg[:, h * 32:(h + 1) * 32]
        wg_aug[h * 32 + 9, h * 32:(h + 1) * 32] = w['b_gat'][h * 32:(h + 1) * 32]
    W3_aug = np.zeros((65, 128), np.float32)
    W3_aug[0:64] = w['W3']
    W3_aug[64] = w['b3']

    def fold(cw, cb, g, be, m, v):
        s = g / np.sqrt(v + 1e-5)
        return cw * s[:, None, None], (cb - m) * s + be

    c1w, c1b = fold(w['conv1_w'], w['conv1_b'], w['bn1_g'], w['bn1_b'],
                    w['bn1_m'], w['bn1_v'])
    c2w, c2b = fold(w['conv2_w'], w['conv2_b'], w['bn2_g'], w['bn2_b'],
                    w['bn2_m'], w['bn2_v'])
    # [cin, k, cout] flattened so slice k -> [cin, cout]
    w1k = np.ascontiguousarray(c1w.transpose(1, 2, 0)).reshape(30, 3 * 64)
    w2k = np.ascontiguousarray(c2w.transpose(1, 2, 0)).reshape(64, 3 * 64)
    fc1_Wr = np.ascontiguousarray(w['fc1_W'].reshape(64, 16 * 64))

    seq = w['seq_data']                              # [B, 30, 20]
    xseq = np.ascontiguousarray(seq.transpose(1, 0, 2)).reshape(30, -1)

    return dict(
        was_aug=was_aug.astype(np.float16), wad_aug=wad_aug, wg_aug=wg_aug,
        W2=w['W2'], b2row=np.ascontiguousarray(np.broadcast_to(w['b2'], (P, 64))),
        W3_aug=W3_aug,
        w1k=w1k, b1=np.ascontiguousarray(c1b.reshape(64, 1)),
        w2k=w2k, b2c=np.ascontiguousarray(c2b.reshape(64, 1)),
        fc1_Wr=fc1_Wr, fc1_b=np.ascontiguousarray(w['fc1_b'].reshape(64, 1)),
        fus_W0=np.ascontiguousarray(w['fus_W'][0:128]),
        fus_W1=np.ascontiguousarray(w['fus_W'][128:192]),
        fus_b=np.ascontiguousarray(w['fus_b'].reshape(1, 128)),
        cls1_W=w['cls1_W'],
        cls1_b=np.ascontiguousarray(w['cls1_b'].reshape(1, 64)),
        cls3_W=w['cls3_W'],
        cls3_b_t=np.array([[float(w['cls3_b'][0])]], np.float32),
        xseq=xseq,
    )


# --------------------------------------------------------------------------
# device program
# --------------------------------------------------------------------------

def build_nc(baked, unroll=False):
    WPC, LOCAL, NTOT, SLOTS = (baked['WPC'], baked['LOCAL'], baked['NTOT'],
                               baked['SLOTS'])
    segs, seg_T = baked['segs'], baked['seg_T']
    Bsz = baked['Bsz']
    BROWS = Bsz + 8
    REALC = baked['REAL']
    RG = [list(range(NC_CORES))]

    nc = bacc.Bacc("TRN2", target_bir_lowering=False, debug=False,
                   num_devices=NC_CORES)

    def inp(name, shape, dt=F32):
        return nc.dram_tensor(name, shape, dt, kind="ExternalInput")

    xslots = inp("xslots", [P, SLOTS * 9], F16)
    xslotsT = inp("xslotsT", [16, SLOTS * P], F16)
    srcrow = inp("srcrow", [P, SLOTS], I32)
    xlocT = inp("xlocT", [16, LOCAL])
    deg_w = inp("deg_w", [P, WPC])
    bl_w = inp("bl_w", [P, WPC])
    cnt_l = inp("cnt_l", [P, 2])
    scat = inp("scat", [P, 2], I32)
    iota256 = inp("iota256", [P, 256])
    ident = inp("ident", [P, P])
    ones4 = inp("ones4", [P, 4])
    onesrow = inp("onesrow", [1, Bsz])
    was_aug = inp("was_aug", [16, 4], F16)
    wad_aug = inp("wad_aug", [16, 4])
    wg_aug = inp("wg_aug", [128, 128])
    W2 = inp("W2", [128, 64])
    b2row = inp("b2row", [P, 64])
    W3_aug = inp("W3_aug", [65, 128])
    w1k = inp("w1k", [30, 3 * 64])
    b1 = inp("b1", [64, 1])
    w2k = inp("w2k", [64, 3 * 64])
    b2c = inp("b2c", [64, 1])
    fc1_Wr = inp("fc1_Wr", [64, 16 * 64])
    fc1_b = inp("fc1_b", [64, 1])
    fus_W0 = inp("fus_W0", [128, 128])
    fus_W1 = inp("fus_W1", [64, 128])
    fus_b = inp("fus_b", [1, 128])
    cls1_W = inp("cls1_W", [128, 64])
    cls1_b = inp("cls1_b", [1, 64])
    cls3_W = inp("cls3_W", [64, 1])
    cls3_b_t = inp("cls3_b_t", [1, 1])
    xseq = inp("xseq", [30, Bsz * 20])

    out = nc.dram_tensor("out", [1, Bsz], F32, kind="ExternalOutput")
    dbg_T2 = nc.dram_tensor("dbg_T2", [LOCAL, 64], F32, kind="ExternalOutput") if DEBUG else None
    dbg_T3 = nc.dram_tensor("dbg_T3", [LOCAL, 64], F32, kind="ExternalOutput") if DEBUG else None
    dbg_AR = nc.dram_tensor("dbg_AR", [BROWS, 128], F32, kind="ExternalOutput") if DEBUG else None
    dbg_sT = nc.dram_tensor("dbg_sT", [64, Bsz], F32, kind="ExternalOutput") if DEBUG else None

    # Tables are partition-major: row (c*LOCAL + p*WPC + w) holds node
    # (core c, window w, partition p); T2_pm is the same buffer viewed
    # [P, WPC*64] so a G-window store is one strided DMA.
    T2_pm = nc.dram_tensor("T2_local", [P, WPC * 64], F16)
    s1_dram = nc.dram_tensor("s1_dram", [64, Bsz * 18], F32)
    s2_dram = nc.dram_tensor("s2_dram", [64, Bsz * 16], F32)
    T2_full = nc.dram_tensor("T2_full", [NTOT, 64], F16)
    T3_pm = nc.dram_tensor("T3_local", [P, WPC * 64], F16)
    T3_full = nc.dram_tensor("T3_full", [NTOT, 64], F16)
    AR_in = nc.dram_tensor("AR_in", [BROWS, 128], F32)
    AR_out = nc.dram_tensor("AR_out", [BROWS, 128], F32)

    def emit_grouped(tc, w0, w1, Gmax, body_group):
        """body_group(wb, G): emit G windows starting at window wb."""
        ngr = (w1 - w0) // Gmax
        if ngr > 0:
            if unroll:
                for wb in range(w0, w0 + ngr * Gmax, Gmax):
                    body_group(wb, Gmax)
            else:
                with tc.For_i(w0, w0 + ngr * Gmax, Gmax) as wb:
                    body_group(wb, Gmax)
        tail = (w1 - w0) % Gmax
        if tail:
            body_group(w0 + ngr * Gmax, tail)

    with tile.TileContext(nc) as tc:
        with tc.tile_pool(name="const", bufs=1) as cp, \
             tc.tile_pool(name="work", bufs=2) as wp, \
             tc.tile_pool(name="gat", bufs=2) as gp, \
             tc.tile_pool(name="psum", bufs=4, space="PSUM") as pp, \
             tc.tile_pool(name="pa", bufs=2, space="PSUM") as pa, \
             tc.tile_pool(name="ppool", bufs=1, space="PSUM") as ppool, \
             tc.tile_pool(name="seq", bufs=1) as sq:

            def c_load(ap, shape, dt=F32):
                t = cp.tile(shape, dt, tag=f"c_{ap.name}")
                nc.sync.dma_start(t[:], ap[:])
                return t

            srcrow_sb = c_load(srcrow, [P, SLOTS], I32)
            deg_sb = c_load(deg_w, [P, WPC])
            bl_sb = c_load(bl_w, [P, WPC])
            cnt_sb = c_load(cnt_l, [P, 2])
            scat_sb = c_load(scat, [P, 2], I32)
            iota_sb = c_load(iota256, [P, 256])
            ident_sb = c_load(ident, [P, P])
            ones4_sb = c_load(ones4, [P, 4])
            was_sb = c_load(was_aug, [16, 4], F16)
            wad_sb = c_load(wad_aug, [16, 4])
            wg_sb = c_load(wg_aug, [128, 128])
            W2_sb = c_load(W2, [128, 64])
            b2row_sb = c_load(b2row, [P, 64])
            W3_sb = c_load(W3_aug, [65, 128])

            dinv_sb = cp.tile([P, WPC], F32)
            nc.scalar.activation(dinv_sb[:], deg_sb[:], AF.Sqrt)
            nc.vector.reciprocal(dinv_sb[:], dinv_sb[:])

            # persistent pooling PSUM, zeroed via K=1 matmul (sets has_written)
            pool_ps0 = ppool.tile([P, P], F32, tag="pool0")
            pool_ps1 = ppool.tile([P, P], F32, tag="pool1")
            zrow = cp.tile([1, P], F32)
            nc.vector.memset(zrow[:], 0.0)
            nc.tensor.matmul(pool_ps0[:], zrow[:], zrow[:], start=True, stop=True)
            nc.tensor.matmul(pool_ps1[:], zrow[:], zrow[:], start=True, stop=True)

            # ================= GAT =================
            def gat_group(wb, G, Gmax, T, seg_col0, w0):
                colb = seg_col0 - w0 * T
                xw = gp.tile([16, Gmax * P], F32, tag="xw")
                nc.scalar.dma_start(xw[:, :G * P], xlocT[:, bass.ds(wb * P, G * P)])
                ad_ps = pa.tile([P, 4 * Gmax], F32, tag="pa")
                for j in range(G):
                    nc.tensor.matmul(ad_ps[:, 4 * j:4 * j + 4],
                                     xw[:, j * P:(j + 1) * P], wad_sb[:],
                                     start=True, stop=True)
                T_w = gp.tile([P, 4 * Gmax], F32, tag="Tw")
                nc.scalar.activation(T_w[:, :4 * G], ad_ps[:, :4 * G],
                                     AF.Exp, scale=0.8)

                XT = gp.tile([16, Gmax * T * P], F16, tag="XT")
                nc.sync.dma_start(
                    XT[:, :G * T * P],
                    xslotsT[:, bass.ds((colb + wb * T) * P, G * T * P)])
                as_ps = pa.tile([P, 4 * Gmax * T], F32, tag="pa")
                for s in range(G * T):
                    nc.tensor.matmul(as_ps[:, 4 * s:4 * s + 4],
                                     XT[:, s * P:(s + 1) * P], was_sb[:],
                                     start=True, stop=True)
                Pt = gp.tile([P, 4 * Gmax * T], F32, tag="Pt")
                Rt = gp.tile([P, 4 * Gmax * T], F32, tag="Rt")
                nc.scalar.activation(Pt[:, :4 * G * T], as_ps[:, :4 * G * T],
                                     AF.Exp, scale=1.0)
                nc.scalar.activation(Rt[:, :4 * G * T], as_ps[:, :4 * G * T],
                                     AF.Exp, scale=0.2)

                XS = gp.tile([P, Gmax * T * 9], F16, tag="XS")
                nc.sync.dma_start(
                    XS[:, :G * T * 9],
                    xslots[:, bass.ds((colb + wb * T) * 9, G * T * 9)])
                T2s_g = gp.tile([P, Gmax * 64], F16, tag="T2s")

                for j in range(G):
                    EX = gp.tile([P, 4 * T], F32, tag="EX")
                    nc.vector.tensor_tensor(
                        EX[:].rearrange("p (t h) -> p t h", h=4),
                        Pt[:, 4 * j * T:4 * (j + 1) * T]
                            .rearrange("p (t h) -> p t h", h=4),
                        T_w[:, 4 * j:4 * j + 4][:, None, :].to_broadcast([P, T, 4]),
                        op=OP.mult)
                    nc.vector.tensor_tensor(EX[:], EX[:],
                                            Rt[:, 4 * j * T:4 * (j + 1) * T],
                                            op=OP.max)
                    S4 = gp.tile([P, 4], F32, tag="S4")
                    nc.vector.tensor_reduce(
                        S4[:, :, None],
                        EX[:].rearrange("p (t h) -> p h t", h=4),
                        axis=AX.X, op=OP.add)
                    nc.vector.reciprocal(S4[:], S4[:])
                    AL = gp.tile([P, 4 * T], F16, tag="AL")
                    nc.vector.tensor_tensor(
                        AL[:].rearrange("p (t h) -> p t h", h=4),
                        EX[:].rearrange("p (t h) -> p t h", h=4),
                        S4[:, None, :].to_broadcast([P, T, 4]),
                        op=OP.mult)

                    ZR = gp.tile([P, T * 36], F16, tag="ZR")
                    nc.vector.tensor_tensor(
                        ZR[:].rearrange("p (t h f) -> p t h f", h=4, f=9),
                        XS[:, j * T * 9:(j + 1) * T * 9]
                            .rearrange("p (t f) -> p t f", f=9)[:, :, None, :]
                            .to_broadcast([P, T, 4, 9]),
                        AL[:].rearrange("p (t h) -> p t h", h=4)[:, :, :, None]
                            .to_broadcast([P, T, 4, 9]),
                        op=OP.mult)
                    zaug = gp.tile([P, 128], F32, tag="zaug")
                    nc.vector.memset(
                        zaug[:].rearrange("p (h t) -> p h t", t=32)[:, :, 10:32], 0.0)
                    nc.vector.tensor_copy(
                        zaug[:].rearrange("p (h t) -> p h t", t=32)[:, :, 9:10],
                        ones4_sb[:, :, None])
                    nc.vector.tensor_reduce(
                        zaug[:].rearrange("p (h t) -> p h t", t=32)[:, :, 0:9][:, :, :, None],
                        ZR[:].rearrange("p (t h f) -> p h f t", h=4, f=9),
                        axis=AX.X, op=OP.add)
                    zT_ps = pp.tile([P, P], F32, tag="ps")
                    nc.tensor.transpose(out=zT_ps[:], in_=zaug[:],
                                        identity=ident_sb[:])
                    zT = gp.tile([P, P], F32, tag="zT")
                    nc.scalar.copy(zT[:], zT_ps[:])
                    g1_ps = pp.tile([P, P], F32, tag="ps")
                    nc.tensor.matmul(g1_ps[:], wg_sb[:], zT[:],
                                     start=True, stop=True)
                    g1T = gp.tile([P, P], F32, tag="g1T")
                    nc.scalar.activation(g1T[:], g1_ps[:], AF.Lrelu, alpha=0.01)
                    h2_ps = pp.tile([P, 64], F32, tag="ps")
                    nc.tensor.matmul(h2_ps[:], g1T[:], W2_sb[:],
                                     start=True, stop=True)
                    nc.scalar.activation(T2s_g[:, j * 64:(j + 1) * 64], h2_ps[:],
                                         AF.Copy,
                                         scale=dinv_sb[:, bass.ds(wb + j, 1)])
                nc.sync.dma_start(T2_pm[:, bass.ds(wb * 64, G * 64)],
                                  T2s_g[:, :G * 64])

            def run_gat():
                seg_col0 = 0
                for (w0, w1), T in zip(segs, seg_T):
                    Gmax = max(1, min(4, 512 // (4 * T)))
                    emit_grouped(
                        tc, w0, w1, Gmax,
                        lambda wb, G, Gmax=Gmax, T=T, s0=seg_col0, w0=w0:
                        gat_group(wb, G, Gmax, T, s0, w0))
                    seg_col0 += (w1 - w0) * T
            run_gat()

            lastw = REALC // P
            pstart = REALC % P
            ztail = wp.tile([P - pstart, 64], F16, tag="ztail")
            nc.vector.memset(ztail[:], 0.0)
            nc.sync.dma_start(T2_pm[pstart:P, bass.ds(lastw * 64, 64)], ztail[:])

            # ================= seq branch (independent of graph; emitted
            # after GAT so it runs during the first AllGather) ==
            w1_sb = c_load(w1k, [30, 3 * 64])
            b1_sb = c_load(b1, [64, 1])
            w2_sb = c_load(w2k, [64, 3 * 64])
            b2c_sb = c_load(b2c, [64, 1])
            fc1_sb = c_load(fc1_Wr, [64, 16 * 64])
            fc1b_sb = c_load(fc1_b, [64, 1])
            fusW0_sb = c_load(fus_W0, [128, 128])
            fusW1_sb = c_load(fus_W1, [64, 128])
            fusb_sb = c_load(fus_b, [1, 128])
            cls1W_sb = c_load(cls1_W, [128, 64])
            cls1b_sb = c_load(cls1_b, [1, 64])
            cls3W_sb = c_load(cls3_W, [64, 1])
            cls3b_sb = c_load(cls3_b_t, [1, 1])
            onesr_sb = c_load(onesrow, [1, Bsz])

            CH1 = 28
            nb1 = (Bsz + CH1 - 1) // CH1
            for ci in range(nb1):
                b0 = ci * CH1
                bn = min(CH1, Bsz - b0)
                xs_ch = sq.tile([30, CH1 * 20], F32, tag="xs_ch")
                nc.sync.dma_start(xs_ch[:30, :bn * 20],
                                  xseq[:, b0 * 20:(b0 + bn) * 20])
                cps = pp.tile([64, CH1 * 18], F32, tag="ps")
                for k in range(3):
                    nc.tensor.matmul(
                        cps[:, :bn * 18],
                        w1_sb[:, 64 * k:64 * (k + 1)],
                        xs_ch[:].rearrange("c (b t) -> c b t", t=20)[:, 0:bn, k:k + 18],
                        start=(k == 0), stop=(k == 2))
                s1c = sq.tile([64, CH1 * 18], F32, tag="s1c")
                nc.scalar.activation(
                    s1c[:, :bn * 18], cps[:, :bn * 18],
                    AF.Lrelu, bias=b1_sb[:], alpha=0.01)
                nc.sync.dma_start(s1_dram[:, b0 * 18:(b0 + bn) * 18],
                                  s1c[:, :bn * 18])
            CH2 = 31
            nb2 = (Bsz + CH2 - 1) // CH2
            for ci in range(nb2):
                b0 = ci * CH2
                bn = min(CH2, Bsz - b0)
                s1c2 = sq.tile([64, CH2 * 18], F32, tag="s1c2")
                nc.sync.dma_start(s1c2[:, :bn * 18],
                                  s1_dram[:, b0 * 18:(b0 + bn) * 18])
                cps2 = pp.tile([64, CH2 * 16], F32, tag="ps")
                for k in range(3):
                    nc.tensor.matmul(
                        cps2[:, :bn * 16],
                        w2_sb[:, 64 * k:64 * (k + 1)],
                        s1c2[:].rearrange("c (b t) -> c b t", t=18)[:, 0:bn, k:k + 16],
                        start=(k == 0), stop=(k == 2))
                s2c = sq.tile([64, CH2 * 16], F32, tag="s2c")
                nc.scalar.activation(
                    s2c[:, :bn * 16], cps2[:, :bn * 16],
                    AF.Lrelu, bias=b2c_sb[:], alpha=0.01)
                nc.sync.dma_start(s2_dram[:, b0 * 16:(b0 + bn) * 16],
                                  s2c[:, :bn * 16])
            sT = sq.tile([64, Bsz], F32, tag="sT")
            for ci in range(Bsz // 512):
                b0 = ci * 512
                s2c3 = sq.tile([64, 512 * 16], F32, tag="s2c3")
                nc.sync.dma_start(s2c3[:], s2_dram[:, b0 * 16:(b0 + 512) * 16])
                fps = pp.tile([64, 512], F32, tag="ps")
                for t in range(16):
                    nc.tensor.matmul(
                        fps[:],
                        fc1_sb[:].rearrange("c (t j) -> c t j", j=64)[:, t, :],
                        s2c3[:].rearrange("c (b t) -> c b t", t=16)[:, :, t:t + 1],
                        start=(t == 0), stop=(t == 15))
                nc.scalar.activation(sT[:, b0:b0 + 512], fps[:],
                                     AF.Identity, bias=fc1b_sb[:])


            nc.gpsimd.collective_compute(
                "AllGather", OP.bypass, replica_groups=RG,
                ins=[T2_pm.ap().opt()], outs=[T2_full.ap().opt()])

            # ================= GCN layers =================
            def gcn_group(wb, G, Gmax, T, seg_col0, w0, table, last):
                colb = seg_col0 - w0 * T
                IDX = wp.tile([P, Gmax * T], I32, tag="IDXw")
                nc.vector.tensor_copy(IDX[:, :G * T],
                                      srcrow_sb[:, bass.ds(colb + wb * T, G * T)])
                Gt = wp.tile([P, Gmax * T * 64], F16, tag="G")
                nc.gpsimd.indirect_dma_start(
                    out=Gt[:, :G * T * 64], out_offset=None,
                    in_=table[:],
                    in_offset=bass.IndirectOffsetOnAxis(
                        ap=IDX[:, 0:G * T], axis=0))
                if not last:
                    T3s_g = wp.tile([P, Gmax * 64], F16, tag="T3s")
                for j in range(G):
                    z = wp.tile([P, 64], F32, tag="z")
                    nc.vector.tensor_reduce(
                        z[:, :, None],
                        Gt[:, j * T * 64:(j + 1) * T * 64]
                            .rearrange("p (t c) -> p c t", c=64),
                        axis=AX.X, op=OP.add)
                    if not last:
                        nc.vector.tensor_scalar(
                            z[:], z[:], dinv_sb[:, bass.ds(wb + j, 1)], None,
                            OP.mult)
                        nc.vector.tensor_tensor(z[:], z[:], b2row_sb[:], op=OP.add)
                        g2 = wp.tile([P, 64], F32, tag="g2")
                        nc.scalar.activation(g2[:], z[:], AF.Lrelu, alpha=0.01)
                        nc.scalar.activation(T3s_g[:, j * 64:(j + 1) * 64], g2[:],
                                             AF.Copy,
                                             scale=dinv_sb[:, bass.ds(wb + j, 1)])
                    else:
                        z3s = wp.tile([P, 65], F32, tag="z3s")
                        nc.scalar.activation(z3s[:, 0:64], z[:], AF.Copy,
                                             scale=dinv_sb[:, bass.ds(wb + j, 1)])
                        nc.vector.tensor_copy(z3s[:, 64:65], ones4_sb[:, 0:1])
                        z3T_ps = pp.tile([65, P], F32, tag="ps")
                        nc.tensor.transpose(out=z3T_ps[:], in_=z3s[:],
                                            identity=ident_sb[:])
                        z3T = wp.tile([65, P], F32, tag="z3T")
                        nc.scalar.copy(z3T[:], z3T_ps[:])
                        g3_ps = pp.tile([P, P], F32, tag="ps")
                        nc.tensor.matmul(g3_ps[:], z3T[:], W3_sb[:],
                                         start=True, stop=True)
                        g3 = wp.tile([P, P], F32, tag="g3")
                        nc.scalar.activation(g3[:], g3_ps[:], AF.Lrelu, alpha=0.01)
                        Mp = wp.tile([P, 256], F32, tag="Mp")
                        nc.vector.tensor_scalar(
                            Mp[:], iota_sb[:], bl_sb[:, bass.ds(wb + j, 1)], None,
                            OP.is_equal)
                        nc.tensor.matmul(pool_ps0[:], Mp[:, 0:128], g3[:],
                                         start=False, stop=True)
                        nc.tensor.matmul(pool_ps1[:], Mp[:, 128:256], g3[:],
                                         start=False, stop=True)
                if not last:
                    nc.scalar.dma_start(T3_pm[:, bass.ds(wb * 64, G * 64)],
                                        T3s_g[:, :G * 64])

            def run_gcn(table, last):
                seg_col0 = 0
                for (w0, w1), T in zip(segs, seg_T):
                    Gmax = max(1, min(4, 512 // (4 * T)))
                    emit_grouped(
                        tc, w0, w1, Gmax,
                        lambda wb, G, Gmax=Gmax, T=T, s0=seg_col0, w0=w0:
                        gcn_group(wb, G, Gmax, T, s0, w0, table, last))
                    seg_col0 += (w1 - w0) * T

            run_gcn(T2_full, False)

            ztail2 = wp.tile([P - pstart, 64], F16, tag="ztail")
            nc.vector.memset(ztail2[:], 0.0)
            nc.sync.dma_start(T3_pm[pstart:P, bass.ds(lastw * 64, 64)], ztail2[:])

            nc.gpsimd.collective_compute(
                "AllGather", OP.bypass, replica_groups=RG,
                ins=[T3_pm.ap().opt()], outs=[T3_full.ap().opt()])

            run_gcn(T3_full, True)

            # ---- pool epilogue
            zb = wp.tile([P, 128], F32, tag="zb")
            nc.vector.memset(zb[:], 0.0)
            r0 = 0
            while r0 < BROWS:
                r1 = min(r0 + P, BROWS)
                nc.sync.dma_start(AR_in[r0:r1, :], zb[:r1 - r0, :])
                r0 = r1
            crec = wp.tile([P, 2], F32, tag="crec")
            nc.vector.reciprocal(crec[:], cnt_sb[:])
            for k, pps in enumerate((pool_ps0, pool_ps1)):
                pooled = wp.tile([P, 128], F32, tag="pooled")
                nc.scalar.activation(pooled[:], pps[:], AF.Copy,
                                     scale=crec[:, k:k + 1])
                nc.gpsimd.indirect_dma_start(
                    out=AR_in[:], out_offset=bass.IndirectOffsetOnAxis(
                        ap=scat_sb[:, k:k + 1], axis=0),
                    in_=pooled[:], in_offset=None)

            nc.gpsimd.collective_compute(
                "AllReduce", OP.add, replica_groups=RG,
                ins=[AR_in.ap().opt()], outs=[AR_out.ap().opt()])

            if DEBUG:
                dtile2 = sq.tile([P, 128], F32, tag="dtile2")
                r0 = 0
                while r0 < BROWS:
                    r1 = min(r0 + P, BROWS)
                    nc.sync.dma_start(dtile2[:r1 - r0, :], AR_out[r0:r1, :])
                    nc.sync.dma_start(dbg_AR[r0:r1, :], dtile2[:r1 - r0, :])
                    r0 = r1
            poolT = sq.tile([P, Bsz], F32, tag="poolT")
            for i in range(Bsz // P):
                blk = sq.tile([P, P], F32, tag="blk")
                nc.sync.dma_start(blk[:], AR_out[i * P:(i + 1) * P, :])
                tp = pp.tile([P, P], F32, tag="ps")
                nc.tensor.transpose(out=tp[:], in_=blk[:], identity=ident_sb[:])
                nc.scalar.copy(poolT[:, i * P:(i + 1) * P], tp[:])

            # ---- fusion + classifier
            combT = sq.tile([P, Bsz], F32, tag="combT")
            for ci in range(Bsz // 512):
                b0 = ci * 512
                ups = pp.tile([P, 512], F32, tag="ps")
                nc.tensor.matmul(ups[:], fusW0_sb[:], poolT[:, b0:b0 + 512],
                                 start=True, stop=False)
                nc.tensor.matmul(ups[:], fusW1_sb[:], sT[:, b0:b0 + 512],
                                 start=False, stop=False)
                nc.tensor.matmul(ups[:], fusb_sb[:], onesr_sb[:, b0:b0 + 512],
                                 start=False, stop=True)
                nc.scalar.activation(combT[:, b0:b0 + 512], ups[:],
                                     AF.Lrelu, alpha=0.01)
            c1T = sq.tile([64, Bsz], F32, tag="c1T")
            for ci in range(Bsz // 512):
                b0 = ci * 512
                vps = pp.tile([64, 512], F32, tag="ps")
                nc.tensor.matmul(vps[:], cls1W_sb[:], combT[:, b0:b0 + 512],
                                 start=True, stop=False)
                nc.tensor.matmul(vps[:], cls1b_sb[:], onesr_sb[:, b0:b0 + 512],
                                 start=False, stop=True)
                nc.scalar.activation(c1T[:, b0:b0 + 512], vps[:],
                                     AF.Lrelu, alpha=0.01)
            out_sb = sq.tile([1, Bsz], F32, tag="out_sb")
            for ci in range(Bsz // 512):
                b0 = ci * 512
                ops_ = pp.tile([1, 512], F32, tag="ps")
                nc.tensor.matmul(ops_[:], cls3W_sb[:], c1T[:, b0:b0 + 512],
                                 start=True, stop=True)
                nc.vector.tensor_scalar(
                    out_sb[:, b0:b0 + 512], ops_[:], cls3b_sb[0:1, 0:1], None,
                    OP.add)
            if DEBUG:
                nc.sync.dma_start(dbg_sT[:], sT[:])
            nc.sync.dma_start(out[:], out_sb[:])

    nc.compile()
    return nc


# --------------------------------------------------------------------------
# entry point
# --------------------------------------------------------------------------

_CACHE = {}
_RUN_KW = {}      # test harness may set e.g. {'trace': True}
_LAST = [None]    # test harness reads BassKernelResults (exec_time_ns)


def kernel(**inputs):
    key = (np.asarray(inputs['edge_index']).tobytes(),)
    kh = hash(key)
    if kh not in _CACHE:
        per_core, baked = host_prep(inputs)
        nc = build_nc(baked)
        _CACHE[kh] = (per_core, baked, nc)
    per_core, baked, nc = _CACHE[kh]

    wts = fold_weights(inputs)
    Bsz = baked['Bsz']
    shared = dict(
        iota256=np.ascontiguousarray(
            np.broadcast_to(np.arange(256, dtype=np.float32), (P, 256))),
        ident=np.eye(P, dtype=np.float32),
        ones4=np.ones((P, 4), np.float32),
        onesrow=np.ones((1, Bsz), np.float32),
        **wts)
    in_maps = []
    for c in range(NC_CORES):
        m = dict(shared)
        m.update(per_core[c])
        in_maps.append(m)

    res = run_bass_kernel_spmd(nc, in_maps, core_ids=list(range(NC_CORES)),
                               **_RUN_KW)
    _LAST[0] = res
    o = res.results[0]["out"].reshape(Bsz, 1).astype(np.float32)
    return o


# revision 23
# speedup vs baseline: 1.6644x; 1.0933x over previous
"""Trainium2 Bass kernel for nn_DeepCPP (GAT + 2xGCN graph branch, conv1d seq
branch, fusion MLP), SPMD over 8 NeuronCores.

Sharding/strategy:
 - Nodes partitioned across cores in natural order (keeps sorted `batch`
   contiguous per core); within a core nodes are sorted by in-degree so
   128-node windows have near-uniform max degree (node-major slot grids),
   processed by segmented hardware loops.
 - GAT layer is gather-free: x[src] per edge slot is materialized host-side,
   attention logits computed on-device per slot-column via small matmuls, and
   exp(leakyrelu(a_s+a_d)) is factorized as max(P_e*T_d, R_e) with
   P=exp(a_s), R=exp(0.2*a_s), T=exp(0.8*a_d); the per-dst factor
   exp(-0.2*a_d) cancels in the softmax.
 - GCN layers gather fp16 rows (dinv-prescaled h) from an AllGathered table
   via one 2D-offset indirect DMA per 128-node window; aggregation is a
   strided vector reduction.
 - Mean-pool via one-hot selection matmuls into persistent PSUM, AllReduce of
   partials; the replicated feature-major seq branch is emitted right after
   GAT so it runs concurrently with the first AllGather. No global barriers:
   Tile's DRAM-tensor dependency tracking orders stores -> collective ->
   gathers, and the collectives sync across cores in hardware.
"""

import sys

sys.path.insert(0, '/opt/trn_rl_repo')

import numpy as np
import ml_dtypes

import concourse.bass as bass
import concourse.mybir as mybir
import concourse.tile as tile
from concourse import bacc
from concourse.bass_utils import run_bass_kernel_spmd

F32 = mybir.dt.float32
F16 = mybir.dt.float16
BF16 = mybir.dt.bfloat16
I32 = mybir.dt.int32
AF = mybir.ActivationFunctionType
OP = mybir.AluOpType
AX = mybir.AxisListType

NC_CORES = 8
P = 128
DEBUG = False


# --------------------------------------------------------------------------
# host-side prep
# --------------------------------------------------------------------------

def _segments(Ts, max_segs=6):
    W = len(Ts)
    INF = float('inf')
    best = [[INF] * (max_segs + 1) for _ in range(W + 1)]
    arg = [[None] * (max_segs + 1) for _ in range(W + 1)]
    best[0][0] = 0.0
    for j in range(1, W + 1):
        for s in range(1, max_segs + 1):
            for i in range(j):
                if best[i][s - 1] == INF:
                    continue
                c = best[i][s - 1] + (j - i) * Ts[i]
                if c < best[j][s]:
                    best[j][s] = c
                    arg[j][s] = i
    s = min(range(1, max_segs + 1), key=lambda k: best[W][k])
    bounds = []
    j = W
    while j > 0:
        i = arg[j][s]
        bounds.append((i, j))
        j = i
        s -= 1
    bounds.reverse()
    return bounds


def host_prep(inputs):
    x = np.asarray(inputs['x'], np.float32)
    ei = np.asarray(inputs['edge_index'], np.int64)
    batch = np.asarray(inputs['batch'], np.int64)
    N = x.shape[0]
    Bsz = int(np.asarray(inputs['seq_data']).shape[0])
    assert N % NC_CORES == 0
    REAL = N // NC_CORES
    WPC = (REAL + P - 1) // P
    LOCAL = WPC * P
    NTOT = LOCAL * NC_CORES
    # Table rows live partition-major: node (core c, window w, partition p)
    # -> row c*LOCAL + p*WPC + w, so a window-group store is one strided DMA.
    assert REAL < LOCAL, "need at least one pad slot for the sentinel row"
    SENT = (P - 1) * WPC + (WPC - 1)            # always-zero pad row, core 0

    src2 = np.concatenate([ei[0], np.arange(N)])
    dst2 = np.concatenate([ei[1], np.arange(N)])
    deg = np.bincount(dst2, minlength=N)

    local_rank = np.zeros(N, np.int64)
    crow = np.zeros(N, np.int64)    # compute-layout row: c*LOCAL + w*P + p
    rowid = np.zeros(N, np.int64)   # table row (partition-major)
    node_at = np.full((NC_CORES, LOCAL), -1, np.int64)
    for c in range(NC_CORES):
        ns = np.arange(c * REAL, (c + 1) * REAL)
        order = ns[np.argsort(-deg[ns], kind='stable')]
        local_rank[order] = np.arange(REAL)
        crow[order] = c * LOCAL + np.arange(REAL)
        rowid[order] = (c * LOCAL + (np.arange(REAL) % P) * WPC
                        + np.arange(REAL) // P)
        node_at[c, :REAL] = order

    Tw = np.ones(WPC, np.int64)
    for c in range(NC_CORES):
        first = node_at[c, ::P]
        for w in range(WPC):
            if first[w] >= 0:
                Tw[w] = max(Tw[w], deg[first[w]])
    segs = _segments([int(t) for t in Tw])
    seg_T = [int(Tw[w0]) for (w0, w1) in segs]
    col_off = np.zeros(WPC, np.int64)
    TW = np.zeros(WPC, np.int64)
    off = 0
    for (w0, w1), T in zip(segs, seg_T):
        for w in range(w0, w1):
            col_off[w] = off + (w - w0) * T
            TW[w] = T
        off += (w1 - w0) * T
    SLOTS = int(off)

    e_dst = crow[dst2]
    e_src = src2
    o = np.argsort(e_dst, kind='stable')
    e_dst = e_dst[o]
    e_src = e_src[o]
    grp_start = np.searchsorted(e_dst, np.arange(NTOT), side='left')
    t_of = np.arange(len(e_dst)) - grp_start[e_dst]
    c_of = e_dst // LOCAL
    lrow = e_dst % LOCAL
    w_of = lrow // P
    p_of = lrow % P
    col = col_off[w_of] + t_of
    assert (t_of < TW[w_of]).all()

    slot_node = np.full((NC_CORES, P, SLOTS), N, np.int64)
    slot_node[c_of, p_of, col] = e_src

    x_pad = np.vstack([x, np.zeros((1, x.shape[1]), np.float32)])
    rowid_pad = np.concatenate([rowid, [SENT]]).astype(np.int32)

    cnt = np.bincount(batch, minlength=Bsz).astype(np.float32)
    per_core = []
    for c in range(NC_CORES):
        sn = slot_node[c]                          # [P, SLOTS], N = pad
        xs = x_pad[sn]                             # [P, SLOTS, 9]
        xslots = np.ascontiguousarray(xs.reshape(P, SLOTS * 9)).astype(np.float16)
        xTl = np.zeros((16, SLOTS, P), np.float32)
        xTl[0:9] = xs.transpose(2, 1, 0)
        xTl[9] = (sn.T == N).astype(np.float32)    # pad flag
        xslotsT = np.ascontiguousarray(xTl.reshape(16, SLOTS * P)).astype(np.float16)
        srcrow = rowid_pad[sn]

        valid = node_at[c] >= 0
        xloc = np.zeros((16, LOCAL), np.float32)
        xloc[0:9, valid] = x[node_at[c][valid]].T

        dg = np.full(LOCAL, 1e30, np.float32)
        dg[valid] = deg[node_at[c][valid]]
        deg_w = np.ascontiguousarray(dg.reshape(WPC, P).T)

        bl = np.full(LOCAL, -1.0, np.float32)
        b_base = int(batch[c * REAL])
        bl[valid] = batch[node_at[c][valid]] - b_base
        assert bl.max() < 256, "batch window exceeded 256"
        bl_w = np.ascontiguousarray(bl.reshape(WPC, P).T)

        cnt_l = np.ones(256, np.float32)
        hi = min(256, Bsz - b_base)
        cnt_l[:hi] = np.maximum(cnt[b_base:b_base + hi], 1.0)
        scatv = np.zeros(256, np.int32)
        for j in range(256):
            scatv[j] = b_base + j if b_base + j < Bsz else Bsz + (j % 8)

        per_core.append(dict(
            xslots=xslots, xslotsT=xslotsT, srcrow=srcrow.astype(np.int32),
            xlocT=xloc, deg_w=deg_w, bl_w=bl_w,
            cnt_l=np.ascontiguousarray(cnt_l.reshape(2, P).T),
            scat=np.ascontiguousarray(scatv.reshape(2, P).T),
        ))

    baked = dict(N=N, REAL=REAL, WPC=WPC, LOCAL=LOCAL, NTOT=NTOT,
                 SLOTS=SLOTS, segs=segs, seg_T=seg_T, Bsz=Bsz)
    return per_core, baked


def fold_weights(inputs):
    w = {k: np.asarray(v, np.float32) for k, v in inputs.items()
         if k not in ('x', 'edge_index', 'batch')}
    H, C = 4, 32
    Wg = w['W_gat']
    was = np.einsum('fhc,hc->fh', Wg.reshape(9, H, C), w['att_src'])
    wad = np.einsum('fhc,hc->fh', Wg.reshape(9, H, C), w['att_dst'])
    was_aug = np.zeros((16, 4), np.float32)
    was_aug[0:9] = was
    was_aug[9] = -80.0
    wad_aug = np.zeros((16, 4), np.float32)
    wad_aug[0:9] = wad
    wg_aug = np.zeros((128, 128), np.float32)
    for h in range(H):
        wg_aug[h * 32:h * 32 + 9, h * 32:(h + 1) * 32] = Wg[:, h * 32:(h + 1) * 32]
        wg_aug[h * 32 + 9, h * 32:(h + 1) * 32] = w['b_gat'][h * 32:(h + 1) * 32]
    W3_aug = np.zeros((65, 128), np.float32)
    W3_aug[0:64] = w['W3']
    W3_aug[64] = w['b3']

    def fold(cw, cb, g, be, m, v):
        s = g / np.sqrt(v + 1e-5)
        return cw * s[:, None, None], (cb - m) * s + be

    c1w, c1b = fold(w['conv1_w'], w['conv1_b'], w['bn1_g'], w['bn1_b'],
                    w['bn1_m'], w['bn1_v'])
    c2w, c2b = fold(w['conv2_w'], w['conv2_b'], w['bn2_g'], w['bn2_b'],
                    w['bn2_m'], w['bn2_v'])
    # [cin, k, cout] flattened so slice k -> [cin, cout]
    w1k = np.ascontiguousarray(c1w.transpose(1, 2, 0)).reshape(30, 3 * 64)
    w2k = np.ascontiguousarray(c2w.transpose(1, 2, 0)).reshape(64, 3 * 64)
    fc1_Wr = np.ascontiguousarray(w['fc1_W'].reshape(64, 16 * 64))

    seq = w['seq_data']                              # [B, 30, 20]
    xseq = np.ascontiguousarray(seq.transpose(1, 0, 2)).reshape(30, -1)

    return dict(
        was_aug=was_aug.astype(np.float16), wad_aug=wad_aug, wg_aug=wg_aug,
        W2=w['W2'], b2row=np.ascontiguousarray(np.broadcast_to(w['b2'], (P, 64))),
        W3_aug=W3_aug,
        w1k=w1k.astype(np.float16),
        b1=np.ascontiguousarray(c1b.reshape(64, 1)),
        w2k=w2k.astype(np.float16),
        b2c=np.ascontiguousarray(c2b.reshape(64, 1)),
        fc1_Wr=fc1_Wr.astype(np.float16),
        fc1_b=np.ascontiguousarray(w['fc1_b'].reshape(64, 1)),
        fus_W0=np.ascontiguousarray(w['fus_W'][0:128]),
        fus_W1=np.ascontiguousarray(w['fus_W'][128:192]),
        fus_b=np.ascontiguousarray(w['fus_b'].reshape(1, 128)),
        cls1_W=w['cls1_W'],
        cls1_b=np.ascontiguousarray(w['cls1_b'].reshape(1, 64)),
        cls3_W=w['cls3_W'],
        cls3_b_t=np.array([[float(w['cls3_b'][0])]], np.float32),
        xseq=xseq.astype(np.float16),
    )


# --------------------------------------------------------------------------
# device program
# --------------------------------------------------------------------------

def build_nc(baked, unroll=False):
    WPC, LOCAL, NTOT, SLOTS = (baked['WPC'], baked['LOCAL'], baked['NTOT'],
                               baked['SLOTS'])
    segs, seg_T = baked['segs'], baked['seg_T']
    Bsz = baked['Bsz']
    BROWS = Bsz + 8
    REALC = baked['REAL']
    RG = [list(range(NC_CORES))]

    nc = bacc.Bacc("TRN2", target_bir_lowering=False, debug=False,
                   num_devices=NC_CORES)

    def inp(name, shape, dt=F32):
        return nc.dram_tensor(name, shape, dt, kind="ExternalInput")

    xslots = inp("xslots", [P, SLOTS * 9], F16)
    xslotsT = inp("xslotsT", [16, SLOTS * P], F16)
    srcrow = inp("srcrow", [P, SLOTS], I32)
    xlocT = inp("xlocT", [16, LOCAL])
    deg_w = inp("deg_w", [P, WPC])
    bl_w = inp("bl_w", [P, WPC])
    cnt_l = inp("cnt_l", [P, 2])
    scat = inp("scat", [P, 2], I32)
    iota256 = inp("iota256", [P, 256])
    ident = inp("ident", [P, P])
    ones4 = inp("ones4", [P, 4])
    onesrow = inp("onesrow", [1, Bsz])
    was_aug = inp("was_aug", [16, 4], F16)
    wad_aug = inp("wad_aug", [16, 4])
    wg_aug = inp("wg_aug", [128, 128])
    W2 = inp("W2", [128, 64])
    b2row = inp("b2row", [P, 64])
    W3_aug = inp("W3_aug", [65, 128])
    w1k = inp("w1k", [30, 3 * 64], F16)
    b1 = inp("b1", [64, 1])
    w2k = inp("w2k", [64, 3 * 64], F16)
    b2c = inp("b2c", [64, 1])
    fc1_Wr = inp("fc1_Wr", [64, 16 * 64], F16)
    fc1_b = inp("fc1_b", [64, 1])
    fus_W0 = inp("fus_W0", [128, 128])
    fus_W1 = inp("fus_W1", [64, 128])
    fus_b = inp("fus_b", [1, 128])
    cls1_W = inp("cls1_W", [128, 64])
    cls1_b = inp("cls1_b", [1, 64])
    cls3_W = inp("cls3_W", [64, 1])
    cls3_b_t = inp("cls3_b_t", [1, 1])
    xseq = inp("xseq", [30, Bsz * 20], F16)

    out = nc.dram_tensor("out", [1, Bsz], F32, kind="ExternalOutput")
    dbg_T2 = nc.dram_tensor("dbg_T2", [LOCAL, 64], F32, kind="ExternalOutput") if DEBUG else None
    dbg_T3 = nc.dram_tensor("dbg_T3", [LOCAL, 64], F32, kind="ExternalOutput") if DEBUG else None
    dbg_AR = nc.dram_tensor("dbg_AR", [BROWS, 128], F32, kind="ExternalOutput") if DEBUG else None
    dbg_sT = nc.dram_tensor("dbg_sT", [64, Bsz], F32, kind="ExternalOutput") if DEBUG else None

    # Tables are partition-major: row (c*LOCAL + p*WPC + w) holds node
    # (core c, window w, partition p); T2_pm is the same buffer viewed
    # [P, WPC*64] so a G-window store is one strided DMA.
    T2_pm = nc.dram_tensor("T2_local", [P, WPC * 64], F16)
    s1_dram = nc.dram_tensor("s1_dram", [64, Bsz * 18], F16)
    s2_dram = nc.dram_tensor("s2_dram", [64, Bsz * 16], F16)
    T2_full = nc.dram_tensor("T2_full", [NTOT, 64], F16)
    T3_pm = nc.dram_tensor("T3_local", [P, WPC * 64], F16)
    T3_full = nc.dram_tensor("T3_full", [NTOT, 64], F16)
    AR_in = nc.dram_tensor("AR_in", [BROWS, 128], F32)
    AR_out = nc.dram_tensor("AR_out", [BROWS, 128], F32)

    def emit_grouped(tc, w0, w1, Gmax, body_group):
        """body_group(wb, G): emit G windows starting at window wb."""
        ngr = (w1 - w0) // Gmax
        if ngr > 0:
            if unroll:
                for wb in range(w0, w0 + ngr * Gmax, Gmax):
                    body_group(wb, Gmax)
            else:
                with tc.For_i(w0, w0 + ngr * Gmax, Gmax) as wb:
                    body_group(wb, Gmax)
        tail = (w1 - w0) % Gmax
        if tail:
            body_group(w0 + ngr * Gmax, tail)

    with tile.TileContext(nc) as tc:
        with tc.tile_pool(name="const", bufs=1) as cp, \
             tc.tile_pool(name="work", bufs=2) as wp, \
             tc.tile_pool(name="gat", bufs=2) as gp, \
             tc.tile_pool(name="psum", bufs=4, space="PSUM") as pp, \
             tc.tile_pool(name="pa", bufs=2, space="PSUM") as pa, \
             tc.tile_pool(name="ppool", bufs=1, space="PSUM") as ppool, \
             tc.tile_pool(name="seq", bufs=1) as sq:

            def c_load(ap, shape, dt=F32):
                t = cp.tile(shape, dt, tag=f"c_{ap.name}")
                nc.sync.dma_start(t[:], ap[:])
                return t

            srcrow_sb = c_load(srcrow, [P, SLOTS], I32)
            deg_sb = c_load(deg_w, [P, WPC])
            bl_sb = c_load(bl_w, [P, WPC])
            cnt_sb = c_load(cnt_l, [P, 2])
            scat_sb = c_load(scat, [P, 2], I32)
            iota_sb = c_load(iota256, [P, 256])
            ident_sb = c_load(ident, [P, P])
            ones4_sb = c_load(ones4, [P, 4])
            was_sb = c_load(was_aug, [16, 4], F16)
            wad_sb = c_load(wad_aug, [16, 4])
            wg_sb = c_load(wg_aug, [128, 128])
            W2_sb = c_load(W2, [128, 64])
            b2row_sb = c_load(b2row, [P, 64])
            W3_sb = c_load(W3_aug, [65, 128])

            dinv_sb = cp.tile([P, WPC], F32)
            nc.scalar.activation(dinv_sb[:], deg_sb[:], AF.Sqrt)
            nc.vector.reciprocal(dinv_sb[:], dinv_sb[:])

            # persistent pooling PSUM, zeroed via K=1 matmul (sets has_written)
            pool_ps0 = ppool.tile([P, P], F32, tag="pool0")
            pool_ps1 = ppool.tile([P, P], F32, tag="pool1")
            zrow = cp.tile([1, P], F32)
            nc.vector.memset(zrow[:], 0.0)
            nc.tensor.matmul(pool_ps0[:], zrow[:], zrow[:], start=True, stop=True)
            nc.tensor.matmul(pool_ps1[:], zrow[:], zrow[:], start=True, stop=True)

            # ================= GAT =================
            def gat_group(wb, G, Gmax, T, seg_col0, w0):
                colb = seg_col0 - w0 * T
                xw = gp.tile([16, Gmax * P], F32, tag="xw")
                nc.scalar.dma_start(xw[:, :G * P], xlocT[:, bass.ds(wb * P, G * P)])
                ad_ps = pa.tile([P, 4 * Gmax], F32, tag="pa")
                for j in range(G):
                    nc.tensor.matmul(ad_ps[:, 4 * j:4 * j + 4],
                                     xw[:, j * P:(j + 1) * P], wad_sb[:],
                                     start=True, stop=True)
                T_w = gp.tile([P, 4 * Gmax], F32, tag="Tw")
                nc.scalar.activation(T_w[:, :4 * G], ad_ps[:, :4 * G],
                                     AF.Exp, scale=0.8)

                XT = gp.tile([16, Gmax * T * P], F16, tag="XT")
                nc.sync.dma_start(
                    XT[:, :G * T * P],
                    xslotsT[:, bass.ds((colb + wb * T) * P, G * T * P)])
                as_ps = pa.tile([P, 4 * Gmax * T], F32, tag="pa")
                for s in range(G * T):
                    nc.tensor.matmul(as_ps[:, 4 * s:4 * s + 4],
                                     XT[:, s * P:(s + 1) * P], was_sb[:],
                                     start=True, stop=True)
                Pt = gp.tile([P, 4 * Gmax * T], F32, tag="Pt")
                Rt = gp.tile([P, 4 * Gmax * T], F32, tag="Rt")
                nc.scalar.activation(Pt[:, :4 * G * T], as_ps[:, :4 * G * T],
                                     AF.Exp, scale=1.0)
                nc.scalar.activation(Rt[:, :4 * G * T], as_ps[:, :4 * G * T],
                                     AF.Exp, scale=0.2)

                XS = gp.tile([P, Gmax * T * 9], F16, tag="XS")
                nc.sync.dma_start(
                    XS[:, :G * T * 9],
                    xslots[:, bass.ds((colb + wb * T) * 9, G * T * 9)])
                T2s_g = gp.tile([P, Gmax * 64], F16, tag="T2s")

                for j in range(G):
                    EX = gp.tile([P, 4 * T], F32, tag="EX")
                    nc.vector.tensor_tensor(
                        EX[:].rearrange("p (t h) -> p t h", h=4),
                        Pt[:, 4 * j * T:4 * (j + 1) * T]
                            .rearrange("p (t h) -> p t h", h=4),
                        T_w[:, 4 * j:4 * j + 4][:, None, :].to_broadcast([P, T, 4]),
                        op=OP.mult)
                    nc.vector.tensor_tensor(EX[:], EX[:],
                                            Rt[:, 4 * j * T:4 * (j + 1) * T],
                                            op=OP.max)
                    S4 = gp.tile([P, 4], F32, tag="S4")
                    nc.vector.tensor_reduce(
                        S4[:, :, None],
                        EX[:].rearrange("p (t h) -> p h t", h=4),
                        axis=AX.X, op=OP.add)
                    nc.vector.reciprocal(S4[:], S4[:])
                    AL = gp.tile([P, 4 * T], F16, tag="AL")
                    nc.vector.tensor_tensor(
                        AL[:].rearrange("p (t h) -> p t h", h=4),
                        EX[:].rearrange("p (t h) -> p t h", h=4),
                        S4[:, None, :].to_broadcast([P, T, 4]),
                        op=OP.mult)

                    ZR = gp.tile([P, T * 36], F16, tag="ZR")
                    nc.vector.tensor_tensor(
                        ZR[:].rearrange("p (t h f) -> p t h f", h=4, f=9),
                        XS[:, j * T * 9:(j + 1) * T * 9]
                            .rearrange("p (t f) -> p t f", f=9)[:, :, None, :]
                            .to_broadcast([P, T, 4, 9]),
                        AL[:].rearrange("p (t h) -> p t h", h=4)[:, :, :, None]
                            .to_broadcast([P, T, 4, 9]),
                        op=OP.mult)
                    zaug = gp.tile([P, 128], F32, tag="zaug")
                    nc.vector.memset(
                        zaug[:].rearrange("p (h t) -> p h t", t=32)[:, :, 10:32], 0.0)
                    nc.vector.tensor_copy(
                        zaug[:].rearrange("p (h t) -> p h t", t=32)[:, :, 9:10],
                        ones4_sb[:, :, None])
                    nc.vector.tensor_reduce(
                        zaug[:].rearrange("p (h t) -> p h t", t=32)[:, :, 0:9][:, :, :, None],
                        ZR[:].rearrange("p (t h f) -> p h f t", h=4, f=9),
                        axis=AX.X, op=OP.add)
                    zT_ps = pp.tile([P, P], F32, tag="ps")
                    nc.tensor.transpose(out=zT_ps[:], in_=zaug[:],
                                        identity=ident_sb[:])
                    zT = gp.tile([P, P], F32, tag="zT")
                    nc.vector.tensor_copy(zT[:], zT_ps[:])
                    g1_ps = pp.tile([P, P], F32, tag="ps")
                    nc.tensor.matmul(g1_ps[:], wg_sb[:], zT[:],
                                     start=True, stop=True)
                    g1T = gp.tile([P, P], F32, tag="g1T")
                    nc.scalar.activation(g1T[:], g1_ps[:], AF.Lrelu, alpha=0.01)
                    h2_ps = pp.tile([P, 64], F32, tag="ps")
                    nc.tensor.matmul(h2_ps[:], g1T[:], W2_sb[:],
                                     start=True, stop=True)
                    nc.scalar.activation(T2s_g[:, j * 64:(j + 1) * 64], h2_ps[:],
                                         AF.Copy,
                                         scale=dinv_sb[:, bass.ds(wb + j, 1)])
                nc.sync.dma_start(T2_pm[:, bass.ds(wb * 64, G * 64)],
                                  T2s_g[:, :G * 64])

            def run_gat():
                seg_col0 = 0
                for (w0, w1), T in zip(segs, seg_T):
                    Gmax = max(1, min(4, 512 // (4 * T)))
                    emit_grouped(
                        tc, w0, w1, Gmax,
                        lambda wb, G, Gmax=Gmax, T=T, s0=seg_col0, w0=w0:
                        gat_group(wb, G, Gmax, T, s0, w0))
                    seg_col0 += (w1 - w0) * T
            run_gat()

            lastw = REALC // P
            pstart = REALC % P
            ztail = wp.tile([P - pstart, 64], F16, tag="ztail")
            nc.vector.memset(ztail[:], 0.0)
            nc.sync.dma_start(T2_pm[pstart:P, bass.ds(lastw * 64, 64)], ztail[:])

            # ================= seq branch (independent of graph; emitted
            # after GAT so it runs during the first AllGather) ==
            w1_sb = c_load(w1k, [30, 3 * 64], F16)
            b1_sb = c_load(b1, [64, 1])
            w2_sb = c_load(w2k, [64, 3 * 64], F16)
            b2c_sb = c_load(b2c, [64, 1])
            fc1_sb = c_load(fc1_Wr, [64, 16 * 64], F16)
            fc1b_sb = c_load(fc1_b, [64, 1])
            fusW0_sb = c_load(fus_W0, [128, 128])
            fusW1_sb = c_load(fus_W1, [64, 128])
            fusb_sb = c_load(fus_b, [1, 128])
            cls1W_sb = c_load(cls1_W, [128, 64])
            cls1b_sb = c_load(cls1_b, [1, 64])
            cls3W_sb = c_load(cls3_W, [64, 1])
            cls3b_sb = c_load(cls3_b_t, [1, 1])
            onesr_sb = c_load(onesrow, [1, Bsz])

            CH1 = 28
            nb1 = (Bsz + CH1 - 1) // CH1
            for ci in range(nb1):
                b0 = ci * CH1
                bn = min(CH1, Bsz - b0)
                xs_ch = sq.tile([30, CH1 * 20], F16, tag="xs_ch")
                nc.sync.dma_start(xs_ch[:30, :bn * 20],
                                  xseq[:, b0 * 20:(b0 + bn) * 20])
                cps = pp.tile([64, CH1 * 18], F32, tag="ps")
                for k in range(3):
                    nc.tensor.matmul(
                        cps[:, :bn * 18],
                        w1_sb[:, 64 * k:64 * (k + 1)],
                        xs_ch[:].rearrange("c (b t) -> c b t", t=20)[:, 0:bn, k:k + 18],
                        start=(k == 0), stop=(k == 2))
                s1c = sq.tile([64, CH1 * 18], F16, tag="s1c")
                nc.scalar.activation(
                    s1c[:, :bn * 18], cps[:, :bn * 18],
                    AF.Lrelu, bias=b1_sb[:], alpha=0.01)
                nc.sync.dma_start(s1_dram[:, b0 * 18:(b0 + bn) * 18],
                                  s1c[:, :bn * 18])


            nc.gpsimd.collective_compute(
                "AllGather", OP.bypass, replica_groups=RG,
                ins=[T2_pm.ap().opt()], outs=[T2_full.ap().opt()])

            # ================= GCN layers =================
            def gcn_group(wb, G, Gmax, T, seg_col0, w0, table, last):
                colb = seg_col0 - w0 * T
                IDX = wp.tile([P, Gmax * T], I32, tag="IDXw")
                nc.vector.tensor_copy(IDX[:, :G * T],
                                      srcrow_sb[:, bass.ds(colb + wb * T, G * T)])
                Gt = wp.tile([P, Gmax * T * 64], F16, tag="G")
                nc.gpsimd.indirect_dma_start(
                    out=Gt[:, :G * T * 64], out_offset=None,
                    in_=table[:],
                    in_offset=bass.IndirectOffsetOnAxis(
                        ap=IDX[:, 0:G * T], axis=0))
                if not last:
                    T3s_g = wp.tile([P, Gmax * 64], F16, tag="T3s")
                for j in range(G):
                    z = wp.tile([P, 64], F32, tag="z")
                    nc.vector.tensor_reduce(
                        z[:, :, None],
                        Gt[:, j * T * 64:(j + 1) * T * 64]
                            .rearrange("p (t c) -> p c t", c=64),
                        axis=AX.X, op=OP.add)
                    if not last:
                        nc.vector.tensor_scalar(
                            z[:], z[:], dinv_sb[:, bass.ds(wb + j, 1)], None,
                            OP.mult)
                        nc.vector.tensor_tensor(z[:], z[:], b2row_sb[:], op=OP.add)
                        g2 = wp.tile([P, 64], F32, tag="g2")
                        nc.scalar.activation(g2[:], z[:], AF.Lrelu, alpha=0.01)
                        nc.scalar.activation(T3s_g[:, j * 64:(j + 1) * 64], g2[:],
                                             AF.Copy,
                                             scale=dinv_sb[:, bass.ds(wb + j, 1)])
                    else:
                        z3s = wp.tile([P, 65], F32, tag="z3s")
                        nc.scalar.activation(z3s[:, 0:64], z[:], AF.Copy,
                                             scale=dinv_sb[:, bass.ds(wb + j, 1)])
                        nc.vector.tensor_copy(z3s[:, 64:65], ones4_sb[:, 0:1])
                        z3T_ps = pp.tile([65, P], F32, tag="ps")
                        nc.tensor.transpose(out=z3T_ps[:], in_=z3s[:],
                                            identity=ident_sb[:])
                        z3T = wp.tile([65, P], F32, tag="z3T")
                        nc.scalar.copy(z3T[:], z3T_ps[:])
                        g3_ps = pp.tile([P, P], F32, tag="ps")
                        nc.tensor.matmul(g3_ps[:], z3T[:], W3_sb[:],
                                         start=True, stop=True)
                        g3 = wp.tile([P, P], F32, tag="g3")
                        nc.scalar.activation(g3[:], g3_ps[:], AF.Lrelu, alpha=0.01)
                        Mp = wp.tile([P, 256], F32, tag="Mp")
                        nc.vector.tensor_scalar(
                            Mp[:], iota_sb[:], bl_sb[:, bass.ds(wb + j, 1)], None,
                            OP.is_equal)
                        nc.tensor.matmul(pool_ps0[:], Mp[:, 0:128], g3[:],
                                         start=False, stop=True)
                        nc.tensor.matmul(pool_ps1[:], Mp[:, 128:256], g3[:],
                                         start=False, stop=True)
                if not last:
                    nc.scalar.dma_start(T3_pm[:, bass.ds(wb * 64, G * 64)],
                                        T3s_g[:, :G * 64])

            def run_gcn(table, last):
                seg_col0 = 0
                for (w0, w1), T in zip(segs, seg_T):
                    Gmax = max(1, min(4, 512 // (4 * T)))
                    emit_grouped(
                        tc, w0, w1, Gmax,
                        lambda wb, G, Gmax=Gmax, T=T, s0=seg_col0, w0=w0:
                        gcn_group(wb, G, Gmax, T, s0, w0, table, last))
                    seg_col0 += (w1 - w0) * T

            run_gcn(T2_full, False)

            ztail2 = wp.tile([P - pstart, 64], F16, tag="ztail")
            nc.vector.memset(ztail2[:], 0.0)
            nc.sync.dma_start(T3_pm[pstart:P, bass.ds(lastw * 64, 64)], ztail2[:])

            # seq part B (conv2 + fc1): runs during AllGather-2
            CH2 = 31
            nb2 = (Bsz + CH2 - 1) // CH2
            for ci in range(nb2):
                b0 = ci * CH2
                bn = min(CH2, Bsz - b0)
                s1c2 = sq.tile([64, CH2 * 18], F16, tag="s1c2")
                nc.sync.dma_start(s1c2[:, :bn * 18],
                                  s1_dram[:, b0 * 18:(b0 + bn) * 18])
                cps2 = pp.tile([64, CH2 * 16], F32, tag="ps")
                for k in range(3):
                    nc.tensor.matmul(
                        cps2[:, :bn * 16],
                        w2_sb[:, 64 * k:64 * (k + 1)],
                        s1c2[:].rearrange("c (b t) -> c b t", t=18)[:, 0:bn, k:k + 16],
                        start=(k == 0), stop=(k == 2))
                s2c = sq.tile([64, CH2 * 16], F16, tag="s2c")
                nc.scalar.activation(
                    s2c[:, :bn * 16], cps2[:, :bn * 16],
                    AF.Lrelu, bias=b2c_sb[:], alpha=0.01)
                nc.sync.dma_start(s2_dram[:, b0 * 16:(b0 + bn) * 16],
                                  s2c[:, :bn * 16])
            sT = sq.tile([64, Bsz], F32, tag="sT")
            for ci in range(Bsz // 512):
                b0 = ci * 512
                s2c3 = sq.tile([64, 512 * 16], F16, tag="s2c3")
                nc.sync.dma_start(s2c3[:], s2_dram[:, b0 * 16:(b0 + 512) * 16])
                fps = pp.tile([64, 512], F32, tag="ps")
                for t in range(16):
                    nc.tensor.matmul(
                        fps[:],
                        fc1_sb[:].rearrange("c (t j) -> c t j", j=64)[:, t, :],
                        s2c3[:].rearrange("c (b t) -> c b t", t=16)[:, :, t:t + 1],
                        start=(t == 0), stop=(t == 15))
                nc.scalar.activation(sT[:, b0:b0 + 512], fps[:],
                                     AF.Identity, bias=fc1b_sb[:])

            nc.gpsimd.collective_compute(
                "AllGather", OP.bypass, replica_groups=RG,
                ins=[T3_pm.ap().opt()], outs=[T3_full.ap().opt()])

            run_gcn(T3_full, True)

            # ---- pool epilogue
            zb = wp.tile([P, 128], F32, tag="zb")
            nc.vector.memset(zb[:], 0.0)
            r0 = 0
            while r0 < BROWS:
                r1 = min(r0 + P, BROWS)
                nc.sync.dma_start(AR_in[r0:r1, :], zb[:r1 - r0, :])
                r0 = r1
            crec = wp.tile([P, 2], F32, tag="crec")
            nc.vector.reciprocal(crec[:], cnt_sb[:])
            for k, pps in enumerate((pool_ps0, pool_ps1)):
                pooled = wp.tile([P, 128], F32, tag="pooled")
                nc.scalar.activation(pooled[:], pps[:], AF.Copy,
                                     scale=crec[:, k:k + 1])
                nc.gpsimd.indirect_dma_start(
                    out=AR_in[:], out_offset=bass.IndirectOffsetOnAxis(
                        ap=scat_sb[:, k:k + 1], axis=0),
                    in_=pooled[:], in_offset=None)

            nc.gpsimd.collective_compute(
                "AllReduce", OP.add, replica_groups=RG,
                ins=[AR_in.ap().opt()], outs=[AR_out.ap().opt()])

            if DEBUG:
                dtile2 = sq.tile([P, 128], F32, tag="dtile2")
                r0 = 0
                while r0 < BROWS:
                    r1 = min(r0 + P, BROWS)
                    nc.sync.dma_start(dtile2[:r1 - r0, :], AR_out[r0:r1, :])
                    nc.sync.dma_start(dbg_AR[r0:r1, :], dtile2[:r1 - r0, :])
                    r0 = r1
            poolT = sq.tile([P, Bsz], F32, tag="poolT")
            for i in range(Bsz // P):
                blk = sq.tile([P, P], F32, tag="blk")
                nc.sync.dma_start(blk[:], AR_out[i * P:(i + 1) * P, :])
                tp = pp.tile([P, P], F32, tag="ps")
                nc.tensor.transpose(out=tp[:], in_=blk[:], identity=ident_sb[:])
                nc.scalar.copy(poolT[:, i * P:(i + 1) * P], tp[:])

            # ---- fusion + classifier
            combT = sq.tile([P, Bsz], F32, tag="combT")
            for ci in range(Bsz // 512):
                b0 = ci * 512
                ups = pp.tile([P, 512], F32, tag="ps")
                nc.tensor.matmul(ups[:], fusW0_sb[:], poolT[:, b0:b0 + 512],
                                 start=True, stop=False)
                nc.tensor.matmul(ups[:], fusW1_sb[:], sT[:, b0:b0 + 512],
                                 start=False, stop=False)
                nc.tensor.matmul(ups[:], fusb_sb[:], onesr_sb[:, b0:b0 + 512],
                                 start=False, stop=True)
                nc.scalar.activation(combT[:, b0:b0 + 512], ups[:],
                                     AF.Lrelu, alpha=0.01)
            c1T = sq.tile([64, Bsz], F32, tag="c1T")
            for ci in range(Bsz // 512):
                b0 = ci * 512
                vps = pp.tile([64, 512], F32, tag="ps")
                nc.tensor.matmul(vps[:], cls1W_sb[:], combT[:, b0:b0 + 512],
                                 start=True, stop=False)
                nc.tensor.matmul(vps[:], cls1b_sb[:], onesr_sb[:, b0:b0 + 512],
                                 start=False, stop=True)
                nc.scalar.activation(c1T[:, b0:b0 + 512], vps[:],
                                     AF.Lrelu, alpha=0.01)
            out_sb = sq.tile([1, Bsz], F32, tag="out_sb")
            for ci in range(Bsz // 512):
                b0 = ci * 512
                ops_ = pp.tile([1, 512], F32, tag="ps")
                nc.tensor.matmul(ops_[:], cls3W_sb[:], c1T[:, b0:b0 + 512],
                                 start=True, stop=True)
                nc.vector.tensor_scalar(
                    out_sb[:, b0:b0 + 512], ops_[:], cls3b_sb[0:1, 0:1], None,
                    OP.add)
            if DEBUG:
                nc.sync.dma_start(dbg_sT[:], sT[:])
            nc.sync.dma_start(out[:], out_sb[:])

    nc.compile()
    return nc


# --------------------------------------------------------------------------
# entry point
# --------------------------------------------------------------------------

_CACHE = {}
_RUN_KW = {}      # test harness may set e.g. {'trace': True}
_LAST = [None]    # test harness reads BassKernelResults (exec_time_ns)


def kernel(**inputs):
    key = (np.asarray(inputs['edge_index']).tobytes(),)
    kh = hash(key)
    if kh not in _CACHE:
        per_core, baked = host_prep(inputs)
        nc = build_nc(baked)
        _CACHE[kh] = (per_core, baked, nc)
    per_core, baked, nc = _CACHE[kh]

    wts = fold_weights(inputs)
    Bsz = baked['Bsz']
    shared = dict(
        iota256=np.ascontiguousarray(
            np.broadcast_to(np.arange(256, dtype=np.float32), (P, 256))),
        ident=np.eye(P, dtype=np.float32),
        ones4=np.ones((P, 4), np.float32),
        onesrow=np.ones((1, Bsz), np.float32),
        **wts)
    in_maps = []
    for c in range(NC_CORES):
        m = dict(shared)
        m.update(per_core[c])
        in_maps.append(m)

    res = run_bass_kernel_spmd(nc, in_maps, core_ids=list(range(NC_CORES)),
                               **_RUN_KW)
    _LAST[0] = res
    o = res.results[0]["out"].reshape(Bsz, 1).astype(np.float32)
    return o


# revision 24
# speedup vs baseline: 1.7208x; 1.0339x over previous
"""Trainium2 Bass kernel for nn_DeepCPP (GAT + 2xGCN graph branch, conv1d seq
branch, fusion MLP), SPMD over 8 NeuronCores.

Sharding/strategy:
 - Nodes partitioned across cores in natural order (keeps sorted `batch`
   contiguous per core); within a core nodes are sorted by in-degree so
   128-node windows have near-uniform max degree (node-major slot grids),
   processed by segmented hardware loops.
 - GAT layer is gather-free: x[src] per edge slot is materialized host-side,
   attention logits computed on-device per slot-column via small matmuls, and
   exp(leakyrelu(a_s+a_d)) is factorized as max(P_e*T_d, R_e) with
   P=exp(a_s), R=exp(0.2*a_s), T=exp(0.8*a_d); the per-dst factor
   exp(-0.2*a_d) cancels in the softmax.
 - GCN layers gather fp16 rows (dinv-prescaled h) from an AllGathered table
   via one 2D-offset indirect DMA per 128-node window; aggregation is a
   strided vector reduction.
 - Mean-pool via one-hot selection matmuls into persistent PSUM, AllReduce of
   partials; the replicated feature-major seq branch is emitted right after
   GAT so it runs concurrently with the first AllGather. No global barriers:
   Tile's DRAM-tensor dependency tracking orders stores -> collective ->
   gathers, and the collectives sync across cores in hardware.
"""

import sys

sys.path.insert(0, '/opt/trn_rl_repo')

import numpy as np
import ml_dtypes

import concourse.bass as bass
import concourse.mybir as mybir
import concourse.tile as tile
from concourse import bacc
from concourse.bass_utils import run_bass_kernel_spmd

F32 = mybir.dt.float32
F16 = mybir.dt.float16
BF16 = mybir.dt.bfloat16
I32 = mybir.dt.int32
AF = mybir.ActivationFunctionType
OP = mybir.AluOpType
AX = mybir.AxisListType

NC_CORES = 8
P = 128
DEBUG = False


# --------------------------------------------------------------------------
# host-side prep
# --------------------------------------------------------------------------

def _segments(Ts, max_segs=6):
    W = len(Ts)
    INF = float('inf')
    best = [[INF] * (max_segs + 1) for _ in range(W + 1)]
    arg = [[None] * (max_segs + 1) for _ in range(W + 1)]
    best[0][0] = 0.0
    for j in range(1, W + 1):
        for s in range(1, max_segs + 1):
            for i in range(j):
                if best[i][s - 1] == INF:
                    continue
                c = best[i][s - 1] + (j - i) * Ts[i]
                if c < best[j][s]:
                    best[j][s] = c
                    arg[j][s] = i
    s = min(range(1, max_segs + 1), key=lambda k: best[W][k])
    bounds = []
    j = W
    while j > 0:
        i = arg[j][s]
        bounds.append((i, j))
        j = i
        s -= 1
    bounds.reverse()
    return bounds


def host_prep(inputs):
    x = np.asarray(inputs['x'], np.float32)
    ei = np.asarray(inputs['edge_index'], np.int64)
    batch = np.asarray(inputs['batch'], np.int64)
    N = x.shape[0]
    Bsz = int(np.asarray(inputs['seq_data']).shape[0])
    assert N % NC_CORES == 0
    REAL = N // NC_CORES
    WPC = (REAL + P - 1) // P
    LOCAL = WPC * P
    NTOT = LOCAL * NC_CORES
    # Table rows live partition-major: node (core c, window w, partition p)
    # -> row c*LOCAL + p*WPC + w, so a window-group store is one strided DMA.
    assert REAL < LOCAL, "need at least one pad slot for the sentinel row"
    SENT = (P - 1) * WPC + (WPC - 1)            # always-zero pad row, core 0

    src2 = np.concatenate([ei[0], np.arange(N)])
    dst2 = np.concatenate([ei[1], np.arange(N)])
    deg = np.bincount(dst2, minlength=N)

    local_rank = np.zeros(N, np.int64)
    crow = np.zeros(N, np.int64)    # compute-layout row: c*LOCAL + w*P + p
    rowid = np.zeros(N, np.int64)   # table row (partition-major)
    node_at = np.full((NC_CORES, LOCAL), -1, np.int64)
    for c in range(NC_CORES):
        ns = np.arange(c * REAL, (c + 1) * REAL)
        order = ns[np.argsort(-deg[ns], kind='stable')]
        local_rank[order] = np.arange(REAL)
        crow[order] = c * LOCAL + np.arange(REAL)
        rowid[order] = (c * LOCAL + (np.arange(REAL) % P) * WPC
                        + np.arange(REAL) // P)
        node_at[c, :REAL] = order

    Tw = np.ones(WPC, np.int64)
    for c in range(NC_CORES):
        first = node_at[c, ::P]
        for w in range(WPC):
            if first[w] >= 0:
                Tw[w] = max(Tw[w], deg[first[w]])
    segs = _segments([int(t) for t in Tw])
    seg_T = [int(Tw[w0]) for (w0, w1) in segs]
    col_off = np.zeros(WPC, np.int64)
    TW = np.zeros(WPC, np.int64)
    off = 0
    for (w0, w1), T in zip(segs, seg_T):
        for w in range(w0, w1):
            col_off[w] = off + (w - w0) * T
            TW[w] = T
        off += (w1 - w0) * T
    SLOTS = int(off)

    e_dst = crow[dst2]
    e_src = src2
    o = np.argsort(e_dst, kind='stable')
    e_dst = e_dst[o]
    e_src = e_src[o]
    grp_start = np.searchsorted(e_dst, np.arange(NTOT), side='left')
    t_of = np.arange(len(e_dst)) - grp_start[e_dst]
    c_of = e_dst // LOCAL
    lrow = e_dst % LOCAL
    w_of = lrow // P
    p_of = lrow % P
    col = col_off[w_of] + t_of
    assert (t_of < TW[w_of]).all()

    slot_node = np.full((NC_CORES, P, SLOTS), N, np.int64)
    slot_node[c_of, p_of, col] = e_src

    x_pad = np.vstack([x, np.zeros((1, x.shape[1]), np.float32)])
    rowid_pad = np.concatenate([rowid, [SENT]]).astype(np.int32)

    cnt = np.bincount(batch, minlength=Bsz).astype(np.float32)
    per_core = []
    for c in range(NC_CORES):
        sn = slot_node[c]                          # [P, SLOTS], N = pad
        xs = x_pad[sn]                             # [P, SLOTS, 9]
        xslots = np.ascontiguousarray(xs.reshape(P, SLOTS * 9)).astype(np.float16)
        xTl = np.zeros((16, SLOTS, P), np.float32)
        xTl[0:9] = xs.transpose(2, 1, 0)
        xTl[9] = (sn.T == N).astype(np.float32)    # pad flag
        xslotsT = np.ascontiguousarray(xTl.reshape(16, SLOTS * P)).astype(np.float16)
        srcrow = rowid_pad[sn]

        valid = node_at[c] >= 0
        xloc = np.zeros((16, LOCAL), np.float32)
        xloc[0:9, valid] = x[node_at[c][valid]].T

        dg = np.full(LOCAL, 1e30, np.float32)
        dg[valid] = deg[node_at[c][valid]]
        deg_w = np.ascontiguousarray(dg.reshape(WPC, P).T)

        bl = np.full(LOCAL, -1.0, np.float32)
        b_base = int(batch[c * REAL])
        bl[valid] = batch[node_at[c][valid]] - b_base
        assert bl.max() < 256, "batch window exceeded 256"
        bl_w = np.ascontiguousarray(bl.reshape(WPC, P).T)

        cnt_l = np.ones(256, np.float32)
        hi = min(256, Bsz - b_base)
        cnt_l[:hi] = np.maximum(cnt[b_base:b_base + hi], 1.0)
        scatv = np.zeros(256, np.int32)
        for j in range(256):
            scatv[j] = b_base + j if b_base + j < Bsz else Bsz + (j % 8)

        per_core.append(dict(
            xslots=xslots, xslotsT=xslotsT, srcrow=srcrow.astype(np.int32),
            xlocT=xloc, deg_w=deg_w, bl_w=bl_w,
            cnt_l=np.ascontiguousarray(cnt_l.reshape(2, P).T),
            scat=np.ascontiguousarray(scatv.reshape(2, P).T),
        ))

    baked = dict(N=N, REAL=REAL, WPC=WPC, LOCAL=LOCAL, NTOT=NTOT,
                 SLOTS=SLOTS, segs=segs, seg_T=seg_T, Bsz=Bsz)
    return per_core, baked


def fold_weights(inputs):
    w = {k: np.asarray(v, np.float32) for k, v in inputs.items()
         if k not in ('x', 'edge_index', 'batch')}
    H, C = 4, 32
    Wg = w['W_gat']
    was = np.einsum('fhc,hc->fh', Wg.reshape(9, H, C), w['att_src'])
    wad = np.einsum('fhc,hc->fh', Wg.reshape(9, H, C), w['att_dst'])
    was_aug = np.zeros((16, 4), np.float32)
    was_aug[0:9] = was
    was_aug[9] = -80.0
    wad_aug = np.zeros((16, 4), np.float32)
    wad_aug[0:9] = wad
    wg_aug = np.zeros((128, 128), np.float32)
    for h in range(H):
        wg_aug[h * 32:h * 32 + 9, h * 32:(h + 1) * 32] = Wg[:, h * 32:(h + 1) * 32]
        wg_aug[h * 32 + 9, h * 32:(h + 1) * 32] = w['b_gat'][h * 32:(h + 1) * 32]
    W3_aug = np.zeros((65, 128), np.float32)
    W3_aug[0:64] = w['W3']
    W3_aug[64] = w['b3']

    def fold(cw, cb, g, be, m, v):
        s = g / np.sqrt(v + 1e-5)
        return cw * s[:, None, None], (cb - m) * s + be

    c1w, c1b = fold(w['conv1_w'], w['conv1_b'], w['bn1_g'], w['bn1_b'],
                    w['bn1_m'], w['bn1_v'])
    c2w, c2b = fold(w['conv2_w'], w['conv2_b'], w['bn2_g'], w['bn2_b'],
                    w['bn2_m'], w['bn2_v'])
    # [cin, k, cout] flattened so slice k -> [cin, cout]
    w1k = np.ascontiguousarray(c1w.transpose(1, 2, 0)).reshape(30, 3 * 64)
    w2k = np.ascontiguousarray(c2w.transpose(1, 2, 0)).reshape(64, 3 * 64)
    fc1_Wr = np.ascontiguousarray(w['fc1_W'].reshape(64, 16 * 64))

    seq = w['seq_data']                              # [B, 30, 20]
    xseq = np.ascontiguousarray(seq.transpose(1, 0, 2)).reshape(30, -1)

    return dict(
        was_aug=was_aug.astype(np.float16), wad_aug=wad_aug, wg_aug=wg_aug,
        W2=w['W2'], b2row=np.ascontiguousarray(np.broadcast_to(w['b2'], (P, 64))),
        W3_aug=W3_aug,
        w1k=w1k.astype(np.float16),
        b1=np.ascontiguousarray(c1b.reshape(64, 1)),
        w2k=w2k.astype(np.float16),
        b2c=np.ascontiguousarray(c2b.reshape(64, 1)),
        fc1_Wr=fc1_Wr.astype(np.float16),
        fc1_b=np.ascontiguousarray(w['fc1_b'].reshape(64, 1)),
        fus_W0=np.ascontiguousarray(w['fus_W'][0:128]),
        fus_W1=np.ascontiguousarray(w['fus_W'][128:192]),
        fus_b=np.ascontiguousarray(w['fus_b'].reshape(1, 128)),
        cls1_W=w['cls1_W'],
        cls1_b=np.ascontiguousarray(w['cls1_b'].reshape(1, 64)),
        cls3_W=w['cls3_W'],
        cls3_b_t=np.array([[float(w['cls3_b'][0])]], np.float32),
        xseq=xseq.astype(np.float16),
    )


# --------------------------------------------------------------------------
# device program
# --------------------------------------------------------------------------

def build_nc(baked, unroll=False):
    WPC, LOCAL, NTOT, SLOTS = (baked['WPC'], baked['LOCAL'], baked['NTOT'],
                               baked['SLOTS'])
    segs, seg_T = baked['segs'], baked['seg_T']
    Bsz = baked['Bsz']
    BROWS = Bsz + 8
    REALC = baked['REAL']
    RG = [list(range(NC_CORES))]

    nc = bacc.Bacc("TRN2", target_bir_lowering=False, debug=False,
                   num_devices=NC_CORES)

    def inp(name, shape, dt=F32):
        return nc.dram_tensor(name, shape, dt, kind="ExternalInput")

    xslots = inp("xslots", [P, SLOTS * 9], F16)
    xslotsT = inp("xslotsT", [16, SLOTS * P], F16)
    srcrow = inp("srcrow", [P, SLOTS], I32)
    xlocT = inp("xlocT", [16, LOCAL])
    deg_w = inp("deg_w", [P, WPC])
    bl_w = inp("bl_w", [P, WPC])
    cnt_l = inp("cnt_l", [P, 2])
    scat = inp("scat", [P, 2], I32)
    iota256 = inp("iota256", [P, 256])
    ident = inp("ident", [P, P])
    ones4 = inp("ones4", [P, 4])
    onesrow = inp("onesrow", [1, Bsz])
    was_aug = inp("was_aug", [16, 4], F16)
    wad_aug = inp("wad_aug", [16, 4])
    wg_aug = inp("wg_aug", [128, 128])
    W2 = inp("W2", [128, 64])
    b2row = inp("b2row", [P, 64])
    W3_aug = inp("W3_aug", [65, 128])
    w1k = inp("w1k", [30, 3 * 64], F16)
    b1 = inp("b1", [64, 1])
    w2k = inp("w2k", [64, 3 * 64], F16)
    b2c = inp("b2c", [64, 1])
    fc1_Wr = inp("fc1_Wr", [64, 16 * 64], F16)
    fc1_b = inp("fc1_b", [64, 1])
    fus_W0 = inp("fus_W0", [128, 128])
    fus_W1 = inp("fus_W1", [64, 128])
    fus_b = inp("fus_b", [1, 128])
    cls1_W = inp("cls1_W", [128, 64])
    cls1_b = inp("cls1_b", [1, 64])
    cls3_W = inp("cls3_W", [64, 1])
    cls3_b_t = inp("cls3_b_t", [1, 1])
    xseq = inp("xseq", [30, Bsz * 20], F16)

    out = nc.dram_tensor("out", [1, Bsz], F32, kind="ExternalOutput")
    dbg_T2 = nc.dram_tensor("dbg_T2", [LOCAL, 64], F32, kind="ExternalOutput") if DEBUG else None
    dbg_T3 = nc.dram_tensor("dbg_T3", [LOCAL, 64], F32, kind="ExternalOutput") if DEBUG else None
    dbg_AR = nc.dram_tensor("dbg_AR", [BROWS, 128], F32, kind="ExternalOutput") if DEBUG else None
    dbg_sT = nc.dram_tensor("dbg_sT", [64, Bsz], F32, kind="ExternalOutput") if DEBUG else None

    # Tables are partition-major: row (c*LOCAL + p*WPC + w) holds node
    # (core c, window w, partition p); T2_pm is the same buffer viewed
    # [P, WPC*64] so a G-window store is one strided DMA.
    T2_pm = nc.dram_tensor("T2_local", [P, WPC * 64], F16)
    s1_dram = nc.dram_tensor("s1_dram", [64, Bsz * 18], F16)
    s2_dram = nc.dram_tensor("s2_dram", [64, Bsz * 16], F16)
    T2_full = nc.dram_tensor("T2_full", [NTOT, 64], F16)
    T3_pm = nc.dram_tensor("T3_local", [P, WPC * 64], F16)
    T3_full = nc.dram_tensor("T3_full", [NTOT, 64], F16)
    AR_in = nc.dram_tensor("AR_in", [BROWS, 128], F32)
    AR_out = nc.dram_tensor("AR_out", [BROWS, 128], F32)

    def emit_grouped(tc, w0, w1, Gmax, body_group):
        """body_group(wb, G): emit G windows starting at window wb."""
        ngr = (w1 - w0) // Gmax
        if ngr > 0:
            if unroll:
                for wb in range(w0, w0 + ngr * Gmax, Gmax):
                    body_group(wb, Gmax)
            else:
                with tc.For_i(w0, w0 + ngr * Gmax, Gmax) as wb:
                    body_group(wb, Gmax)
        tail = (w1 - w0) % Gmax
        if tail:
            body_group(w0 + ngr * Gmax, tail)

    with tile.TileContext(nc) as tc:
        with tc.tile_pool(name="const", bufs=1) as cp, \
             tc.tile_pool(name="work", bufs=3) as wp, \
             tc.tile_pool(name="gat", bufs=2) as gp, \
             tc.tile_pool(name="psum", bufs=4, space="PSUM") as pp, \
             tc.tile_pool(name="pa", bufs=2, space="PSUM") as pa, \
             tc.tile_pool(name="ppool", bufs=1, space="PSUM") as ppool, \
             tc.tile_pool(name="seq", bufs=1) as sq:

            def c_load(ap, shape, dt=F32):
                t = cp.tile(shape, dt, tag=f"c_{ap.name}")
                nc.sync.dma_start(t[:], ap[:])
                return t

            srcrow_sb = c_load(srcrow, [P, SLOTS], I32)
            deg_sb = c_load(deg_w, [P, WPC])
            bl_sb = c_load(bl_w, [P, WPC])
            cnt_sb = c_load(cnt_l, [P, 2])
            scat_sb = c_load(scat, [P, 2], I32)
            iota_sb = c_load(iota256, [P, 256])
            ident_sb = c_load(ident, [P, P])
            ones4_sb = c_load(ones4, [P, 4])
            was_sb = c_load(was_aug, [16, 4], F16)
            wad_sb = c_load(wad_aug, [16, 4])
            wg_sb = c_load(wg_aug, [128, 128])
            W2_sb = c_load(W2, [128, 64])
            b2row_sb = c_load(b2row, [P, 64])
            W3_sb = c_load(W3_aug, [65, 128])

            dinv_sb = cp.tile([P, WPC], F32)
            nc.scalar.activation(dinv_sb[:], deg_sb[:], AF.Sqrt)
            nc.vector.reciprocal(dinv_sb[:], dinv_sb[:])

            # persistent pooling PSUM, zeroed via K=1 matmul (sets has_written)
            pool_ps0 = ppool.tile([P, P], F32, tag="pool0")
            pool_ps1 = ppool.tile([P, P], F32, tag="pool1")
            zrow = cp.tile([1, P], F32)
            nc.vector.memset(zrow[:], 0.0)
            nc.tensor.matmul(pool_ps0[:], zrow[:], zrow[:], start=True, stop=True)
            nc.tensor.matmul(pool_ps1[:], zrow[:], zrow[:], start=True, stop=True)

            # ================= GAT =================
            def gat_group(wb, G, Gmax, T, seg_col0, w0):
                colb = seg_col0 - w0 * T
                xw = gp.tile([16, Gmax * P], F32, tag="xw")
                nc.scalar.dma_start(xw[:, :G * P], xlocT[:, bass.ds(wb * P, G * P)])
                ad_ps = pa.tile([P, 4 * Gmax], F32, tag="pa")
                for j in range(G):
                    nc.tensor.matmul(ad_ps[:, 4 * j:4 * j + 4],
                                     xw[:, j * P:(j + 1) * P], wad_sb[:],
                                     start=True, stop=True)
                T_w = gp.tile([P, 4 * Gmax], F32, tag="Tw")
                nc.scalar.activation(T_w[:, :4 * G], ad_ps[:, :4 * G],
                                     AF.Exp, scale=0.8)

                XT = gp.tile([16, Gmax * T * P], F16, tag="XT")
                nc.sync.dma_start(
                    XT[:, :G * T * P],
                    xslotsT[:, bass.ds((colb + wb * T) * P, G * T * P)])
                as_ps = pa.tile([P, 4 * Gmax * T], F32, tag="pa")
                for s in range(G * T):
                    nc.tensor.matmul(as_ps[:, 4 * s:4 * s + 4],
                                     XT[:, s * P:(s + 1) * P], was_sb[:],
                                     start=True, stop=True)
                Pt = gp.tile([P, 4 * Gmax * T], F32, tag="Pt")
                Rt = gp.tile([P, 4 * Gmax * T], F32, tag="Rt")
                nc.scalar.activation(Pt[:, :4 * G * T], as_ps[:, :4 * G * T],
                                     AF.Exp, scale=1.0)
                nc.scalar.activation(Rt[:, :4 * G * T], as_ps[:, :4 * G * T],
                                     AF.Exp, scale=0.2)

                XS = gp.tile([P, Gmax * T * 9], F16, tag="XS")
                nc.sync.dma_start(
                    XS[:, :G * T * 9],
                    xslots[:, bass.ds((colb + wb * T) * 9, G * T * 9)])
                T2s_g = gp.tile([P, Gmax * 64], F16, tag="T2s")

                for j in range(G):
                    EX = gp.tile([P, 4 * T], F32, tag="EX")
                    nc.vector.tensor_tensor(
                        EX[:].rearrange("p (t h) -> p t h", h=4),
                        Pt[:, 4 * j * T:4 * (j + 1) * T]
                            .rearrange("p (t h) -> p t h", h=4),
                        T_w[:, 4 * j:4 * j + 4][:, None, :].to_broadcast([P, T, 4]),
                        op=OP.mult)
                    nc.vector.tensor_tensor(EX[:], EX[:],
                                            Rt[:, 4 * j * T:4 * (j + 1) * T],
                                            op=OP.max)
                    S4 = gp.tile([P, 4], F32, tag="S4")
                    nc.vector.tensor_reduce(
                        S4[:, :, None],
                        EX[:].rearrange("p (t h) -> p h t", h=4),
                        axis=AX.X, op=OP.add)
                    nc.vector.reciprocal(S4[:], S4[:])
                    AL = gp.tile([P, 4 * T], F16, tag="AL")
                    nc.vector.tensor_tensor(
                        AL[:].rearrange("p (t h) -> p t h", h=4),
                        EX[:].rearrange("p (t h) -> p t h", h=4),
                        S4[:, None, :].to_broadcast([P, T, 4]),
                        op=OP.mult)

                    ZR = gp.tile([P, T * 36], F16, tag="ZR")
                    nc.vector.tensor_tensor(
                        ZR[:].rearrange("p (t h f) -> p t h f", h=4, f=9),
                        XS[:, j * T * 9:(j + 1) * T * 9]
                            .rearrange("p (t f) -> p t f", f=9)[:, :, None, :]
                            .to_broadcast([P, T, 4, 9]),
                        AL[:].rearrange("p (t h) -> p t h", h=4)[:, :, :, None]
                            .to_broadcast([P, T, 4, 9]),
                        op=OP.mult)
                    zaug = gp.tile([P, 128], F32, tag="zaug")
                    nc.vector.memset(
                        zaug[:].rearrange("p (h t) -> p h t", t=32)[:, :, 10:32], 0.0)
                    nc.vector.tensor_copy(
                        zaug[:].rearrange("p (h t) -> p h t", t=32)[:, :, 9:10],
                        ones4_sb[:, :, None])
                    nc.vector.tensor_reduce(
                        zaug[:].rearrange("p (h t) -> p h t", t=32)[:, :, 0:9][:, :, :, None],
                        ZR[:].rearrange("p (t h f) -> p h f t", h=4, f=9),
                        axis=AX.X, op=OP.add)
                    zT_ps = pp.tile([P, P], F32, tag="ps")
                    nc.tensor.transpose(out=zT_ps[:], in_=zaug[:],
                                        identity=ident_sb[:])
                    zT = gp.tile([P, P], F32, tag="zT")
                    nc.vector.tensor_copy(zT[:], zT_ps[:])
                    g1_ps = pp.tile([P, P], F32, tag="ps")
                    nc.tensor.matmul(g1_ps[:], wg_sb[:], zT[:],
                                     start=True, stop=True)
                    g1T = gp.tile([P, P], F32, tag="g1T")
                    nc.scalar.activation(g1T[:], g1_ps[:], AF.Lrelu, alpha=0.01)
                    h2_ps = pp.tile([P, 64], F32, tag="ps")
                    nc.tensor.matmul(h2_ps[:], g1T[:], W2_sb[:],
                                     start=True, stop=True)
                    nc.scalar.activation(T2s_g[:, j * 64:(j + 1) * 64], h2_ps[:],
                                         AF.Copy,
                                         scale=dinv_sb[:, bass.ds(wb + j, 1)])
                nc.sync.dma_start(T2_pm[:, bass.ds(wb * 64, G * 64)],
                                  T2s_g[:, :G * 64])

            def run_gat():
                seg_col0 = 0
                for (w0, w1), T in zip(segs, seg_T):
                    Gmax = max(1, min(4, 512 // (4 * T)))
                    emit_grouped(
                        tc, w0, w1, Gmax,
                        lambda wb, G, Gmax=Gmax, T=T, s0=seg_col0, w0=w0:
                        gat_group(wb, G, Gmax, T, s0, w0))
                    seg_col0 += (w1 - w0) * T
            run_gat()

            lastw = REALC // P
            pstart = REALC % P
            ztail = wp.tile([P - pstart, 64], F16, tag="ztail")
            nc.vector.memset(ztail[:], 0.0)
            nc.sync.dma_start(T2_pm[pstart:P, bass.ds(lastw * 64, 64)], ztail[:])

            # ================= seq branch (independent of graph; emitted
            # after GAT so it runs during the first AllGather) ==
            w1_sb = c_load(w1k, [30, 3 * 64], F16)
            b1_sb = c_load(b1, [64, 1])
            w2_sb = c_load(w2k, [64, 3 * 64], F16)
            b2c_sb = c_load(b2c, [64, 1])
            fc1_sb = c_load(fc1_Wr, [64, 16 * 64], F16)
            fc1b_sb = c_load(fc1_b, [64, 1])
            fusW0_sb = c_load(fus_W0, [128, 128])
            fusW1_sb = c_load(fus_W1, [64, 128])
            fusb_sb = c_load(fus_b, [1, 128])
            cls1W_sb = c_load(cls1_W, [128, 64])
            cls1b_sb = c_load(cls1_b, [1, 64])
            cls3W_sb = c_load(cls3_W, [64, 1])
            cls3b_sb = c_load(cls3_b_t, [1, 1])
            onesr_sb = c_load(onesrow, [1, Bsz])

            CH1 = 28
            nb1 = (Bsz + CH1 - 1) // CH1
            for ci in range(nb1):
                b0 = ci * CH1
                bn = min(CH1, Bsz - b0)
                xs_ch = sq.tile([30, CH1 * 20], F16, tag="xs_ch")
                nc.sync.dma_start(xs_ch[:30, :bn * 20],
                                  xseq[:, b0 * 20:(b0 + bn) * 20])
                cps = pp.tile([64, CH1 * 18], F32, tag="ps")
                for k in range(3):
                    nc.tensor.matmul(
                        cps[:, :bn * 18],
                        w1_sb[:, 64 * k:64 * (k + 1)],
                        xs_ch[:].rearrange("c (b t) -> c b t", t=20)[:, 0:bn, k:k + 18],
                        start=(k == 0), stop=(k == 2))
                s1c = sq.tile([64, CH1 * 18], F16, tag="s1c")
                nc.scalar.activation(
                    s1c[:, :bn * 18], cps[:, :bn * 18],
                    AF.Lrelu, bias=b1_sb[:], alpha=0.01)
                nc.sync.dma_start(s1_dram[:, b0 * 18:(b0 + bn) * 18],
                                  s1c[:, :bn * 18])


            nc.gpsimd.collective_compute(
                "AllGather", OP.bypass, replica_groups=RG,
                ins=[T2_pm.ap().opt()], outs=[T2_full.ap().opt()])

            # ================= GCN layers =================
            def gcn_group(wb, G, Gmax, T, seg_col0, w0, table, last):
                colb = seg_col0 - w0 * T
                IDX = wp.tile([P, Gmax * T], I32, tag="IDXw")
                nc.vector.tensor_copy(IDX[:, :G * T],
                                      srcrow_sb[:, bass.ds(colb + wb * T, G * T)])
                Gt = wp.tile([P, Gmax * T * 64], F16, tag="G")
                nc.gpsimd.indirect_dma_start(
                    out=Gt[:, :G * T * 64], out_offset=None,
                    in_=table[:],
                    in_offset=bass.IndirectOffsetOnAxis(
                        ap=IDX[:, 0:G * T], axis=0))
                if not last:
                    T3s_g = wp.tile([P, Gmax * 64], F16, tag="T3s")
                for j in range(G):
                    z = wp.tile([P, 64], F32, tag="z")
                    nc.vector.tensor_reduce(
                        z[:, :, None],
                        Gt[:, j * T * 64:(j + 1) * T * 64]
                            .rearrange("p (t c) -> p c t", c=64),
                        axis=AX.X, op=OP.add)
                    if not last:
                        nc.vector.tensor_scalar(
                            z[:], z[:], dinv_sb[:, bass.ds(wb + j, 1)], None,
                            OP.mult)
                        nc.vector.tensor_tensor(z[:], z[:], b2row_sb[:], op=OP.add)
                        g2 = wp.tile([P, 64], F32, tag="g2")
                        nc.scalar.activation(g2[:], z[:], AF.Lrelu, alpha=0.01)
                        nc.scalar.activation(T3s_g[:, j * 64:(j + 1) * 64], g2[:],
                                             AF.Copy,
                                             scale=dinv_sb[:, bass.ds(wb + j, 1)])
                    else:
                        z3s = wp.tile([P, 65], F32, tag="z3s")
                        nc.scalar.activation(z3s[:, 0:64], z[:], AF.Copy,
                                             scale=dinv_sb[:, bass.ds(wb + j, 1)])
                        nc.vector.tensor_copy(z3s[:, 64:65], ones4_sb[:, 0:1])
                        z3T_ps = pp.tile([65, P], F32, tag="ps")
                        nc.tensor.transpose(out=z3T_ps[:], in_=z3s[:],
                                            identity=ident_sb[:])
                        z3T = wp.tile([65, P], F32, tag="z3T")
                        nc.scalar.copy(z3T[:], z3T_ps[:])
                        g3_ps = pp.tile([P, P], F32, tag="ps")
                        nc.tensor.matmul(g3_ps[:], z3T[:], W3_sb[:],
                                         start=True, stop=True)
                        g3 = wp.tile([P, P], F32, tag="g3")
                        nc.scalar.activation(g3[:], g3_ps[:], AF.Lrelu, alpha=0.01)
                        Mp = wp.tile([P, 256], F32, tag="Mp")
                        nc.vector.tensor_scalar(
                            Mp[:], iota_sb[:], bl_sb[:, bass.ds(wb + j, 1)], None,
                            OP.is_equal)
                        nc.tensor.matmul(pool_ps0[:], Mp[:, 0:128], g3[:],
                                         start=False, stop=True)
                        nc.tensor.matmul(pool_ps1[:], Mp[:, 128:256], g3[:],
                                         start=False, stop=True)
                if not last:
                    nc.scalar.dma_start(T3_pm[:, bass.ds(wb * 64, G * 64)],
                                        T3s_g[:, :G * 64])

            def run_gcn(table, last):
                seg_col0 = 0
                for (w0, w1), T in zip(segs, seg_T):
                    Gmax = max(1, min(4, 512 // (4 * T)))
                    emit_grouped(
                        tc, w0, w1, Gmax,
                        lambda wb, G, Gmax=Gmax, T=T, s0=seg_col0, w0=w0:
                        gcn_group(wb, G, Gmax, T, s0, w0, table, last))
                    seg_col0 += (w1 - w0) * T

            run_gcn(T2_full, False)

            ztail2 = wp.tile([P - pstart, 64], F16, tag="ztail")
            nc.vector.memset(ztail2[:], 0.0)
            nc.sync.dma_start(T3_pm[pstart:P, bass.ds(lastw * 64, 64)], ztail2[:])

            # seq part B (conv2 + fc1): runs during AllGather-2
            CH2 = 31
            nb2 = (Bsz + CH2 - 1) // CH2
            for ci in range(nb2):
                b0 = ci * CH2
                bn = min(CH2, Bsz - b0)
                s1c2 = sq.tile([64, CH2 * 18], F16, tag="s1c2")
                nc.sync.dma_start(s1c2[:, :bn * 18],
                                  s1_dram[:, b0 * 18:(b0 + bn) * 18])
                cps2 = pp.tile([64, CH2 * 16], F32, tag="ps")
                for k in range(3):
                    nc.tensor.matmul(
                        cps2[:, :bn * 16],
                        w2_sb[:, 64 * k:64 * (k + 1)],
                        s1c2[:].rearrange("c (b t) -> c b t", t=18)[:, 0:bn, k:k + 16],
                        start=(k == 0), stop=(k == 2))
                s2c = sq.tile([64, CH2 * 16], F16, tag="s2c")
                nc.scalar.activation(
                    s2c[:, :bn * 16], cps2[:, :bn * 16],
                    AF.Lrelu, bias=b2c_sb[:], alpha=0.01)
                nc.sync.dma_start(s2_dram[:, b0 * 16:(b0 + bn) * 16],
                                  s2c[:, :bn * 16])
            sT = sq.tile([64, Bsz], F32, tag="sT")
            for ci in range(Bsz // 512):
                b0 = ci * 512
                s2c3 = sq.tile([64, 512 * 16], F16, tag="s2c3")
                nc.sync.dma_start(s2c3[:], s2_dram[:, b0 * 16:(b0 + 512) * 16])
                fps = pp.tile([64, 512], F32, tag="ps")
                for t in range(16):
                    nc.tensor.matmul(
                        fps[:],
                        fc1_sb[:].rearrange("c (t j) -> c t j", j=64)[:, t, :],
                        s2c3[:].rearrange("c (b t) -> c b t", t=16)[:, :, t:t + 1],
                        start=(t == 0), stop=(t == 15))
                nc.scalar.activation(sT[:, b0:b0 + 512], fps[:],
                                     AF.Identity, bias=fc1b_sb[:])

            nc.gpsimd.collective_compute(
                "AllGather", OP.bypass, replica_groups=RG,
                ins=[T3_pm.ap().opt()], outs=[T3_full.ap().opt()])

            run_gcn(T3_full, True)

            # ---- pool epilogue
            zb = wp.tile([P, 128], F32, tag="zb")
            nc.vector.memset(zb[:], 0.0)
            r0 = 0
            while r0 < BROWS:
                r1 = min(r0 + P, BROWS)
                nc.sync.dma_start(AR_in[r0:r1, :], zb[:r1 - r0, :])
                r0 = r1
            crec = wp.tile([P, 2], F32, tag="crec")
            nc.vector.reciprocal(crec[:], cnt_sb[:])
            for k, pps in enumerate((pool_ps0, pool_ps1)):
                pooled = wp.tile([P, 128], F32, tag="pooled")
                nc.scalar.activation(pooled[:], pps[:], AF.Copy,
                                     scale=crec[:, k:k + 1])
                nc.gpsimd.indirect_dma_start(
                    out=AR_in[:], out_offset=bass.IndirectOffsetOnAxis(
                        ap=scat_sb[:, k:k + 1], axis=0),
                    in_=pooled[:], in_offset=None)

            nc.gpsimd.collective_compute(
                "AllReduce", OP.add, replica_groups=RG,
                ins=[AR_in.ap().opt()], outs=[AR_out.ap().opt()])

            if DEBUG:
                dtile2 = sq.tile([P, 128], F32, tag="dtile2")
                r0 = 0
                while r0 < BROWS:
                    r1 = min(r0 + P, BROWS)
                    nc.sync.dma_start(dtile2[:r1 - r0, :], AR_out[r0:r1, :])
                    nc.sync.dma_start(dbg_AR[r0:r1, :], dtile2[:r1 - r0, :])
                    r0 = r1
            poolT = sq.tile([P, Bsz], F32, tag="poolT")
            for i in range(Bsz // P):
                blk = sq.tile([P, P], F32, tag="blk")
                nc.sync.dma_start(blk[:], AR_out[i * P:(i + 1) * P, :])
                tp = pp.tile([P, P], F32, tag="ps")
                nc.tensor.transpose(out=tp[:], in_=blk[:], identity=ident_sb[:])
                nc.scalar.copy(poolT[:, i * P:(i + 1) * P], tp[:])

            # ---- fusion + classifier
            combT = sq.tile([P, Bsz], F32, tag="combT")
            for ci in range(Bsz // 512):
                b0 = ci * 512
                ups = pp.tile([P, 512], F32, tag="ps")
                nc.tensor.matmul(ups[:], fusW0_sb[:], poolT[:, b0:b0 + 512],
                                 start=True, stop=False)
                nc.tensor.matmul(ups[:], fusW1_sb[:], sT[:, b0:b0 + 512],
                                 start=False, stop=False)
                nc.tensor.matmul(ups[:], fusb_sb[:], onesr_sb[:, b0:b0 + 512],
                                 start=False, stop=True)
                nc.scalar.activation(combT[:, b0:b0 + 512], ups[:],
                                     AF.Lrelu, alpha=0.01)
            c1T = sq.tile([64, Bsz], F32, tag="c1T")
            for ci in range(Bsz // 512):
                b0 = ci * 512
                vps = pp.tile([64, 512], F32, tag="ps")
                nc.tensor.matmul(vps[:], cls1W_sb[:], combT[:, b0:b0 + 512],
                                 start=True, stop=False)
                nc.tensor.matmul(vps[:], cls1b_sb[:], onesr_sb[:, b0:b0 + 512],
                                 start=False, stop=True)
                nc.scalar.activation(c1T[:, b0:b0 + 512], vps[:],
                                     AF.Lrelu, alpha=0.01)
            out_sb = sq.tile([1, Bsz], F32, tag="out_sb")
            for ci in range(Bsz // 512):
                b0 = ci * 512
                ops_ = pp.tile([1, 512], F32, tag="ps")
                nc.tensor.matmul(ops_[:], cls3W_sb[:], c1T[:, b0:b0 + 512],
                                 start=True, stop=True)
                nc.vector.tensor_scalar(
                    out_sb[:, b0:b0 + 512], ops_[:], cls3b_sb[0:1, 0:1], None,
                    OP.add)
            if DEBUG:
                nc.sync.dma_start(dbg_sT[:], sT[:])
            nc.sync.dma_start(out[:], out_sb[:])

    nc.compile()
    return nc


# --------------------------------------------------------------------------
# entry point
# --------------------------------------------------------------------------

_CACHE = {}
_RUN_KW = {}      # test harness may set e.g. {'trace': True}
_LAST = [None]    # test harness reads BassKernelResults (exec_time_ns)


def kernel(**inputs):
    key = (np.asarray(inputs['edge_index']).tobytes(),)
    kh = hash(key)
    if kh not in _CACHE:
        per_core, baked = host_prep(inputs)
        nc = build_nc(baked)
        _CACHE[kh] = (per_core, baked, nc)
    per_core, baked, nc = _CACHE[kh]

    wts = fold_weights(inputs)
    Bsz = baked['Bsz']
    shared = dict(
        iota256=np.ascontiguousarray(
            np.broadcast_to(np.arange(256, dtype=np.float32), (P, 256))),
        ident=np.eye(P, dtype=np.float32),
        ones4=np.ones((P, 4), np.float32),
        onesrow=np.ones((1, Bsz), np.float32),
        **wts)
    in_maps = []
    for c in range(NC_CORES):
        m = dict(shared)
        m.update(per_core[c])
        in_maps.append(m)

    res = run_bass_kernel_spmd(nc, in_maps, core_ids=list(range(NC_CORES)),
                               **_RUN_KW)
    _LAST[0] = res
    o = res.results[0]["out"].reshape(Bsz, 1).astype(np.float32)
    return o
